# revision 10
# baseline (speedup 1.0000x reference)
"""Trainium2 Bass kernel: MeshGraphNet-style GNN message passing (v2).

Strategy (8 NeuronCores, SPMD, edges partitioned by dst block):
  - Sort edges by dst; nodes in 128-blocks; each core owns a contiguous
    range of 49 blocks and all edges targeting them.
  - Per-step node state h lives in DRAM twice: a per-core fp32
    feature-major copy (residual-precision master, ping-pong A/B) and a
    replicated fp16 node-major gather table (AllGathered each step).
  - h[src] is fetched with dma_gather(transpose=True): 256B fp16 rows,
    delivered ALREADY feature-major - no PE transposes.  The int16 index
    limit (32767 < N) is dodged by splitting each block's edge list into
    a low-src part (src < 32768) and a high-src part, each padded to a
    uniform per-block boundary so the For_i body stays static; the high
    gather reads a row-offset slice of the same table.
  - h[dst] never materializes: pre1 += A_nm @ O_T where A_nm = h@W1d per
    own node (one matmul/block) and O_T is the host-precomputed one-hot
    (fp16, loaded per block-step).
  - The edge encoder's last layer is folded into the per-step W1e
    weights, so only relu2 (penultimate activation) is stored.
  - scatter-mean via one-hot matmul accumulation in PSUM (O_scat built
    on DVE from the dloc blob); mean + masked edge-bias as vector ops.
  - All matmuls in fp16 (full PE rate), PSUM accumulate fp32.
"""

import os
import numpy as np

P = 128
HALF = 32768  # int16 gather index limit

LAST = {}


def _ceil_div(a, b):
    return -(-a // b)


def _strips(T):
    out = []
    t0 = 0
    while t0 < T:
        k = min(4, T - t0)
        out.append((t0, k))
        t0 += k
    return out


def prep_host(inputs, n_cores=8):
    x = np.asarray(inputs["x"], np.float32)
    ea = np.asarray(inputs["edge_attr"], np.float32)
    ei = np.asarray(inputs["edge_index"], np.int32)
    N, NI = x.shape
    E, EI = ea.shape
    L = np.asarray(inputs["ne_W1"]).shape[1]
    OD = np.asarray(inputs["de_W3"]).shape[1]
    S = np.asarray(inputs["pe_W1"]).shape[0]

    NB = _ceil_div(N, P)
    NB = _ceil_div(NB, n_cores) * n_cores
    BPC = NB // n_cores
    N_pad = NB * P

    src = ei[0].astype(np.int64)
    dst = ei[1].astype(np.int64)
    perm = np.argsort(dst, kind="stable")
    src_s = src[perm].astype(np.int32)
    dst_s = dst[perm].astype(np.int32)
    ea_s = ea[perm]

    deg = np.bincount(dst, minlength=N_pad).astype(np.float32)
    inv_deg = (1.0 / np.maximum(deg, 1.0)).astype(np.float32)
    mask = (deg > 0).astype(np.float32)

    block_start = np.searchsorted(dst_s, np.arange(0, N_pad + 1, P))

    # split each block's edges into low-src / high-src parts
    lo_idx, hi_idx = [], []
    for b in range(NB):
        s0, s1 = int(block_start[b]), int(block_start[b + 1])
        sl = src_s[s0:s1]
        lo_idx.append(np.nonzero(sl < HALF)[0] + s0)
        hi_idx.append(np.nonzero(sl >= HALF)[0] + s0)
    KLo = max(P, max(_ceil_div(len(v), P) for v in lo_idx) * P)
    KHi = max(P, max(_ceil_div(len(v), P) for v in hi_idx) * P)
    E_blk = KLo + KHi
    T = E_blk // P
    C = 2 + T  # blob cols: inv_deg, mask, dloc[p, t]

    blob = np.zeros((NB, P, C), np.float32)
    blob[:, :, 0] = inv_deg.reshape(NB, P)
    blob[:, :, 1] = mask.reshape(NB, P)
    idx16 = np.zeros((NB, P, E_blk // 16), np.int16)
    ot = np.zeros((NB, P, E_blk), np.float16)
    ea_pack = np.zeros((NB, E_blk, EI), np.float32)
    for b in range(NB):
        li, hi = lo_idx[b], hi_idx[b]
        nl, nh = len(li), len(hi)
        dloc = np.full(E_blk, -1.0, np.float32)
        dloc[:nl] = (dst_s[li] - b * P).astype(np.float32)
        dloc[KLo:KLo + nh] = (dst_s[hi] - b * P).astype(np.float32)
        blob[b, :, 2:] = dloc.reshape(T, P).T
        gidx = np.zeros(E_blk, np.int64)
        gidx[:nl] = src_s[li]
        gidx[KLo:KLo + nh] = src_s[hi] - HALF
        w = gidx.reshape(-1, 16).astype(np.int16)  # [E_blk//16, 16]
        idx16[b] = np.tile(w.T, (P // 16, 1))  # [P, E_blk//16]
        ecols = np.arange(E_blk)
        valid = dloc >= 0
        ot[b, :, :] = (dloc[None, :] ==
                       np.arange(P, dtype=np.float32)[:, None]).astype(
                           np.float16)
        ot[b, :, ~valid] = 0
        ea_pack[b, :nl] = ea_s[li]
        ea_pack[b, KLo:KLo + nh] = ea_s[hi]

    x_fm = np.zeros((NI, N_pad), np.float16)
    x_fm[:, :N] = x.T.astype(np.float16)

    params = dict(N=N, NI=NI, E=E, EI=EI, L=L, OD=OD, S=S,
                  NB=NB, BPC=BPC, N_pad=N_pad, T=T, E_blk=E_blk, C=C,
                  KLo=KLo, KHi=KHi, n_cores=n_cores)

    def f16(a):
        return np.ascontiguousarray(np.asarray(a, np.float32).astype(
            np.float16))

    def f32(a):
        return np.ascontiguousarray(np.asarray(a, np.float32))

    ee_W3 = np.asarray(inputs["ee_W3"], np.float32)
    ee_b3 = np.asarray(inputs["ee_b3"], np.float32)
    pe_W1 = np.asarray(inputs["pe_W1"], np.float32)  # [S, 3L, L]
    pe_b1 = np.asarray(inputs["pe_b1"], np.float32)  # [S, L]

    weights = {
        "ne_W1": f16(inputs["ne_W1"]), "ne_W2": f16(inputs["ne_W2"]),
        "ne_W3": f16(inputs["ne_W3"]),
        "ee_W1": f16(inputs["ee_W1"]), "ee_W2": f16(inputs["ee_W2"]),
        "de_W1": f16(inputs["de_W1"]), "de_W2": f16(inputs["de_W2"]),
        "de_W3": f16(inputs["de_W3"]),
        "ne_b1": f32(inputs["ne_b1"]).reshape(L, 1),
        "ne_b2": f32(inputs["ne_b2"]).reshape(L, 1),
        "ne_b3": f32(inputs["ne_b3"]).reshape(L, 1),
        "ee_b1": f32(inputs["ee_b1"]).reshape(L, 1),
        "ee_b2": f32(inputs["ee_b2"]).reshape(L, 1),
        "de_b1": f32(inputs["de_b1"]).reshape(L, 1),
        "de_b2": f32(inputs["de_b2"]).reshape(L, 1),
        "de_b3": f16(np.asarray(inputs["de_b3"], np.float32).reshape(1, OD)),
        "pe_W1d": f16(pe_W1[:, 0:L, :].reshape(S * L, L)),
        "pe_W1s": f16(pe_W1[:, L:2 * L, :].reshape(S * L, L)),
        "pe_W1e": f16(np.einsum("kl,slm->skm", ee_W3,
                                pe_W1[:, 2 * L:3 * L, :]).reshape(S * L, L)),
        "pe_W2": f16(inputs["pe_W2"]).reshape(S * L, L),
        "pe_W3": f16(inputs["pe_W3"]).reshape(S * L, L),
        "pn_W1h": f16(np.asarray(inputs["pn_W1"],
                                 np.float32)[:, 0:L, :].reshape(S * L, L)),
        "pn_W1a": f16(np.asarray(inputs["pn_W1"],
                                 np.float32)[:, L:2 * L, :].reshape(S * L, L)),
        "pn_W2": f16(inputs["pn_W2"]).reshape(S * L, L),
        "pn_W3": f16(inputs["pn_W3"]).reshape(S * L, L),
        "pe_b1": f32(pe_b1 + np.einsum("l,slm->sm", ee_b3.reshape(L),
                                       pe_W1[:, 2 * L:3 * L, :])).reshape(
                                           S * L, 1),
        "pe_b2": f32(inputs["pe_b2"]).reshape(S * L, 1),
        "pe_b3": f16(inputs["pe_b3"]).reshape(S, L),
        "pn_b1": f32(inputs["pn_b1"]).reshape(S * L, 1),
        "pn_b2": f32(inputs["pn_b2"]).reshape(S * L, 1),
        "pn_b3": f32(inputs["pn_b3"]).reshape(S * L, 1),
    }

    in_maps = []
    for c in range(n_cores):
        b0, b1 = c * BPC, (c + 1) * BPC
        m = dict(weights)
        m["xfm"] = np.ascontiguousarray(x_fm[:, b0 * P:b1 * P])
        m["blob"] = np.ascontiguousarray(blob[b0:b1].reshape(BPC * P, C))
        m["idx"] = np.ascontiguousarray(
            idx16[b0:b1].reshape(BPC * P, E_blk // 16))
        m["ot"] = np.ascontiguousarray(ot[b0:b1].reshape(BPC * P, E_blk))
        m["eafm"] = np.ascontiguousarray(
            ea_pack[b0:b1].reshape(BPC * E_blk, EI).T.astype(np.float16))
        in_maps.append(m)
    return params, in_maps


def build_program(params, debug=False):
    import concourse.bass as bass
    import concourse.bacc as bacc
    import concourse.mybir as mybir
    import concourse.tile as tile
    from concourse.bass import ds, ts
    from concourse.masks import make_identity
    from contextlib import ExitStack

    f32 = mybir.dt.float32
    f16 = mybir.dt.float16
    i16 = mybir.dt.int16
    Relu = mybir.ActivationFunctionType.Relu
    Copy = mybir.ActivationFunctionType.Copy
    AO = mybir.AluOpType

    NI, EI, L, OD, S = (params[k] for k in ("NI", "EI", "L", "OD", "S"))
    BPC, N_pad, T, E_blk, C = (params[k] for k in
                               ("BPC", "N_pad", "T", "E_blk", "C"))
    KLo, KHi = params["KLo"], params["KHi"]
    n_cores = params["n_cores"]
    strips = _strips(T)

    nc = bacc.Bacc(None, target_bir_lowering=False, debug=debug,
                   num_swdge_queues=1,
                   dynamic_dma_scratch_size=32768)

    def par(name, shape, dtype=f32, out=False):
        return nc.declare_dram_parameter(name, list(shape), dtype, isOutput=out)

    xfm_d = par("xfm", [NI, BPC * P], f16)
    eafm_d = par("eafm", [EI, BPC * E_blk], f16)
    blob_d = par("blob", [BPC * P, C])
    idx_d = par("idx", [BPC * P, E_blk // 16], i16)
    ot_d = par("ot", [BPC * P, E_blk], f16)

    w_d = {}
    for nm, shp, dt in [
        ("ne_W1", [NI, L], f16), ("ne_W2", [L, L], f16), ("ne_W3", [L, L], f16),
        ("ee_W1", [EI, L], f16), ("ee_W2", [L, L], f16),
        ("de_W1", [L, L], f16), ("de_W2", [L, L], f16), ("de_W3", [L, OD], f16),
        ("ne_b1", [L, 1], f32), ("ne_b2", [L, 1], f32), ("ne_b3", [L, 1], f32),
        ("ee_b1", [L, 1], f32), ("ee_b2", [L, 1], f32),
        ("de_b1", [L, 1], f32), ("de_b2", [L, 1], f32), ("de_b3", [1, OD], f16),
        ("pe_W1d", [S * L, L], f16), ("pe_W1s", [S * L, L], f16),
        ("pe_W1e", [S * L, L], f16), ("pe_W2", [S * L, L], f16),
        ("pe_W3", [S * L, L], f16),
        ("pn_W1h", [S * L, L], f16), ("pn_W1a", [S * L, L], f16),
        ("pn_W2", [S * L, L], f16), ("pn_W3", [S * L, L], f16),
        ("pe_b1", [S * L, 1], f32), ("pe_b2", [S * L, 1], f32),
        ("pe_b3", [S, L], f16),
        ("pn_b1", [S * L, 1], f32), ("pn_b2", [S * L, 1], f32),
        ("pn_b3", [S * L, 1], f32),
    ]:
        w_d[nm] = par(nm, shp, dt)

    out_d = par("out", [BPC * P, OD], out=True)

    # fp16 node-major gather tables (ping-pong), fp32 fm h master (ping-pong)
    h_own = nc.dram_tensor("h_own", [BPC * P, L], f16)
    h_tblA = nc.dram_tensor("h_tblA", [N_pad, L], f16, addr_space="Shared")
    h_tblB = nc.dram_tensor("h_tblB", [N_pad, L], f16, addr_space="Shared")
    h_tbl = [h_tblA, h_tblB]
    h_fmA = nc.dram_tensor("h_fmA", [BPC * P, L], f32)
    h_fmB = nc.dram_tensor("h_fmB", [BPC * P, L], f32)
    h_fm = [h_fmA, h_fmB]
    relu2_d = nc.dram_tensor("relu2", [BPC * P, E_blk], f16)

    with tile.TileContext(nc) as tc, ExitStack() as ctx:
        wp = ctx.enter_context(tc.tile_pool(name="wp", bufs=1))
        sbh = ctx.enter_context(tc.tile_pool(name="sbh", bufs=3))
        sbg = ctx.enter_context(tc.tile_pool(name="sbg", bufs=3))
        sbo = ctx.enter_context(tc.tile_pool(name="sbo", bufs=3))
        sbr = ctx.enter_context(tc.tile_pool(name="sbr", bufs=3))
        sbm = ctx.enter_context(tc.tile_pool(name="sbm", bufs=3))
        sba = ctx.enter_context(tc.tile_pool(name="sba", bufs=3))
        sbs = ctx.enter_context(tc.tile_pool(name="sbs", bufs=2))
        sbn = ctx.enter_context(tc.tile_pool(name="sbn", bufs=3))
        ps_b = ctx.enter_context(tc.tile_pool(name="ps_b", bufs=3,
                                              space="PSUM"))
        ps_a = ctx.enter_context(tc.tile_pool(name="ps_a", bufs=2,
                                              space="PSUM"))
        ps_s = ctx.enter_context(tc.tile_pool(name="ps_s", bufs=3,
                                              space="PSUM"))

        identity = wp.tile([P, P], f32, tag="identity")
        make_identity(nc, identity[:])
        iota_i = wp.tile([P, P], mybir.dt.int32, tag="iota_i")
        nc.gpsimd.iota(iota_i[:], pattern=[[1, P]], base=0,
                       channel_multiplier=0)
        iota_f = wp.tile([P, P], f32, tag="iota_f")
        nc.vector.tensor_copy(iota_f[:], iota_i[:])
        ones_row = wp.tile([1, P], f16, tag="ones_row")
        nc.vector.memset(ones_row[:], 1.0)

        W = {}

        def load(nm, dram_ap, shape, tag, dt=f16):
            t = wp.tile(list(shape), dt, tag=tag)
            nc.sync.dma_start(out=t[:], in_=dram_ap)
            W[nm] = t
            return t

        for nm, shp in [("ne_W1", [NI, L]), ("ne_W2", [L, L]),
                        ("ne_W3", [L, L]), ("ee_W1", [EI, L]),
                        ("ee_W2", [L, L]), ("de_W1", [L, L]),
                        ("de_W2", [L, L]), ("de_W3", [L, OD])]:
            load(nm, w_d[nm][:, :], shp, nm)
        for nm in ("ne_b1", "ne_b2", "ne_b3", "ee_b1", "ee_b2",
                   "de_b1", "de_b2"):
            load(nm, w_d[nm][:, :], [L, 1], nm, f32)
        load("de_b3", w_d["de_b3"][:, :], [1, OD], "de_b3", f16)
        for s in range(S):
            for nm in ("pe_W1d", "pe_W1s", "pe_W1e", "pe_W2", "pe_W3",
                       "pn_W1h", "pn_W1a", "pn_W2", "pn_W3"):
                load(f"{nm}_{s}", w_d[nm][s * L:(s + 1) * L, :], [L, L],
                     f"{nm}_{s}")
            for nm in ("pe_b1", "pe_b2", "pn_b1", "pn_b2", "pn_b3"):
                load(f"{nm}_{s}", w_d[nm][s * L:(s + 1) * L, :], [L, 1],
                     f"{nm}_{s}", f32)
            load(f"pe_b3_{s}", w_d["pe_b3"][s:s + 1, :], [1, L],
                 f"pe_b3_{s}")

        mm = nc.tensor.matmul

        # ---- node encoder: h0 for own nodes -> h_fmA (f32) + table ----
        with tc.For_i(0, BPC, 1, staggered_reset=True) as b:
            x_t = sbn.tile([NI, P], f16, tag="x_t")
            nc.sync.dma_start(out=x_t[:], in_=xfm_d[:, ts(b, P)])
            p1 = ps_s.tile([P, P], f32, tag="ps_small")
            mm(out=p1[:], lhsT=W["ne_W1"][:], rhs=x_t[:], start=True,
               stop=True)
            a1 = sbn.tile([P, P], f16, tag="ne_a1")
            nc.scalar.activation(out=a1[:], in_=p1[:], func=Relu,
                                 bias=W["ne_b1"][:, :1])
            p2 = ps_s.tile([P, P], f32, tag="ps_small")
            mm(out=p2[:], lhsT=W["ne_W2"][:], rhs=a1[:], start=True,
               stop=True)
            a2 = sbn.tile([P, P], f16, tag="ne_a2")
            nc.vector.tensor_scalar(out=a2[:], in0=p2[:],
                                    scalar1=W["ne_b2"][:, :1], scalar2=0.0,
                                    op0=AO.add, op1=AO.max)
            p3 = ps_s.tile([P, P], f32, tag="ps_small")
            mm(out=p3[:], lhsT=W["ne_W3"][:], rhs=a2[:], start=True,
               stop=True)
            h0 = sbn.tile([P, P], f32, tag="ne_h0")
            nc.vector.tensor_scalar(out=h0[:], in0=p3[:],
                                    scalar1=W["ne_b3"][:, :1], scalar2=None,
                                    op0=AO.add)
            nc.sync.dma_start(out=h_fmA[ts(b, P), :], in_=h0[:])
            trp = ps_s.tile([P, P], f32, tag="ps_small")
            nc.tensor.transpose(out=trp[:], in_=h0[:], identity=identity[:])
            hn16 = sbn.tile([P, P], f16, tag="ne_hn16")
            nc.scalar.activation(out=hn16[:], in_=trp[:], func=Copy)
            nc.scalar.dma_start(out=h_own[ts(b, P), :], in_=hn16[:])
        nc.gpsimd.collective_compute(
            "AllGather", mybir.AluOpType.bypass,
            replica_groups=[list(range(n_cores))],
            ins=[h_own[:, :]], outs=[h_tblA[:, :]])

        # ---- edge encoder: relu2 for own edges -> relu2_d (fp16) ----
        with tc.For_i(0, BPC, 1, staggered_reset=True) as b:
            ea_t = sbg.tile([EI, E_blk], f16, tag="ea_t")
            nc.sync.dma_start(out=ea_t[:], in_=eafm_d[:, ts(b, E_blk)])
            r2_all = sbr.tile([P, E_blk], f16, tag="r2_all")
            for (t0, k) in strips:
                w = k * P
                cs = slice(t0 * P, t0 * P + w)
                p1 = ps_b.tile([P, 512], f32, tag="mm_big")
                mm(out=p1[:, :w], lhsT=W["ee_W1"][:], rhs=ea_t[:, cs],
                   start=True, stop=True)
                a1 = sba.tile([P, 512], f16, tag="ee_a1")
                nc.scalar.activation(out=a1[:, :w], in_=p1[:, :w], func=Relu,
                                     bias=W["ee_b1"][:, :1])
                p2 = ps_b.tile([P, 512], f32, tag="mm_big")
                mm(out=p2[:, :w], lhsT=W["ee_W2"][:], rhs=a1[:, :w],
                   start=True, stop=True)
                nc.vector.tensor_scalar(out=r2_all[:, cs], in0=p2[:, :w],
                                        scalar1=W["ee_b2"][:, :1], scalar2=0.0,
                                        op0=AO.add, op1=AO.max)
            nc.scalar.dma_start(out=relu2_d[ts(b, P), :], in_=r2_all[:])

        # ---- message passing steps ----
        for s in range(S):
            rd, wr = s % 2, (s + 1) % 2
            b3p = ps_s.tile([P, L], f32, tag="ps_small")
            mm(out=b3p[:], lhsT=ones_row[:], rhs=W[f"pe_b3_{s}"][:],
               start=True, stop=True)
            b3b = wp.tile([P, L], f32, tag=f"b3b_{s}")
            nc.vector.tensor_copy(b3b[:], b3p[:])

            with tc.For_i(0, BPC, 1, staggered_reset=True) as b:
                blob_t = sbm.tile([P, C], f32, tag="blob_t")
                nc.sync.dma_start(out=blob_t[:], in_=blob_d[ts(b, P), :])
                idx_t = sbm.tile([P, E_blk // 16], i16, tag="idx_t")
                nc.sync.dma_start(out=idx_t[:], in_=idx_d[ts(b, P), :])
                hfm_t = sbh.tile([P, P], f32, tag="hfm_t")
                nc.sync.dma_start(out=hfm_t[:], in_=h_fm[rd][ts(b, P), :])
                ot_t = sbo.tile([P, E_blk], f16, tag="ot_t")
                nc.scalar.dma_start(out=ot_t[:], in_=ot_d[ts(b, P), :])
                r2_t = sbr.tile([P, E_blk], f16, tag="r2_t")
                nc.scalar.dma_start(out=r2_t[:], in_=relu2_d[ts(b, P), :])

                hs3 = sbg.tile([P, 1, E_blk], f16, tag="hs3")
                CH = 512  # SWDGE descriptor-ring capacity limit per inst
                for c0 in range(0, KLo, CH):
                    w = min(CH, KLo - c0)
                    nc.gpsimd.dma_gather(
                        out_ap=hs3[:, :, c0:c0 + w], in_ap=h_tbl[rd][:, :],
                        idxs_ap=idx_t[:, c0 // 16:(c0 + w) // 16],
                        num_idxs=w, num_idxs_reg=w, elem_size=L,
                        transpose=True)
                for c0 in range(0, KHi, CH):
                    w = min(CH, KHi - c0)
                    nc.gpsimd.dma_gather(
                        out_ap=hs3[:, :, KLo + c0:KLo + c0 + w],
                        in_ap=h_tbl[rd][ds(HALF, N_pad - HALF), :],
                        idxs_ap=idx_t[:, (KLo + c0) // 16:(KLo + c0 + w) // 16],
                        num_idxs=w, num_idxs_reg=w, elem_size=L,
                        transpose=True)

                h16 = sbh.tile([P, P], f16, tag="h16")
                nc.scalar.activation(out=h16[:], in_=hfm_t[:], func=Copy)
                ap_nm = ps_s.tile([P, P], f32, tag="ps_small")
                mm(out=ap_nm[:], lhsT=h16[:], rhs=W[f"pe_W1d_{s}"][:],
                   start=True, stop=True)
                a16 = sbh.tile([P, P], f16, tag="a16")
                nc.scalar.activation(out=a16[:], in_=ap_nm[:], func=Copy)

                osc = sbs.tile([P, T * P], f16, tag="osc")
                for t in range(T):
                    nc.vector.tensor_tensor(
                        out=osc[:, t * P:(t + 1) * P],
                        in0=blob_t[:, 2 + t:3 + t].to_broadcast([P, P])[:],
                        in1=iota_f[:], op=AO.is_equal)

                aggp = ps_a.tile([P, L], f32, tag="aggp")
                for si, (t0, k) in enumerate(strips):
                    w = k * P
                    cs = slice(t0 * P, t0 * P + w)
                    pre1 = ps_b.tile([P, 512], f32, tag="mm_big")
                    mm(out=pre1[:, :w], lhsT=a16[:], rhs=ot_t[:, cs],
                       start=True, stop=False)
                    mm(out=pre1[:, :w], lhsT=W[f"pe_W1s_{s}"][:],
                       rhs=hs3[:, 0, cs], start=False, stop=False)
                    mm(out=pre1[:, :w], lhsT=W[f"pe_W1e_{s}"][:],
                       rhs=r2_t[:, cs], start=False, stop=True)
                    a1 = sba.tile([P, 512], f16, tag="pe_a1")
                    if si % 2 == 0:
                        nc.scalar.activation(out=a1[:, :w], in_=pre1[:, :w],
                                             func=Relu,
                                             bias=W[f"pe_b1_{s}"][:, :1])
                    else:
                        nc.vector.tensor_scalar(out=a1[:, :w], in0=pre1[:, :w],
                                                scalar1=W[f"pe_b1_{s}"][:, :1],
                                                scalar2=0.0, op0=AO.add,
                                                op1=AO.max)
                    a2p = ps_b.tile([P, 512], f32, tag="mm_big")
                    mm(out=a2p[:, :w], lhsT=W[f"pe_W2_{s}"][:], rhs=a1[:, :w],
                       start=True, stop=True)
                    a2 = sba.tile([P, 512], f16, tag="pe_a2")
                    if si % 2 == 1:
                        nc.scalar.activation(out=a2[:, :w], in_=a2p[:, :w],
                                             func=Relu,
                                             bias=W[f"pe_b2_{s}"][:, :1])
                    else:
                        nc.vector.tensor_scalar(out=a2[:, :w], in0=a2p[:, :w],
                                                scalar1=W[f"pe_b2_{s}"][:, :1],
                                                scalar2=0.0, op0=AO.add,
                                                op1=AO.max)
                    m_all = ps_b.tile([P, 512], f32, tag="mm_big")
                    for j in range(k):
                        mm(out=m_all[:, j * P:(j + 1) * P],
                           lhsT=a2[:, j * P:(j + 1) * P],
                           rhs=W[f"pe_W3_{s}"][:], start=True, stop=True)
                    m16 = sba.tile([P, 512], f16, tag="m16")
                    if si % 2 == 0:
                        nc.vector.tensor_copy(m16[:, :w], m_all[:, :w])
                    else:
                        nc.scalar.activation(out=m16[:, :w], in_=m_all[:, :w],
                                             func=Copy)
                    for j in range(k):
                        t = t0 + j
                        mm(out=aggp[:], lhsT=osc[:, t * P:(t + 1) * P],
                           rhs=m16[:, j * P:(j + 1) * P],
                           start=(t == 0), stop=(t == T - 1))

                agg_sb = sbh.tile([P, L], f32, tag="agg_sb")
                nc.vector.tensor_scalar(out=agg_sb[:], in0=aggp[:],
                                        scalar1=blob_t[:, 0:1], scalar2=None,
                                        op0=AO.mult)
                b3m = sbh.tile([P, L], f32, tag="b3m")
                nc.vector.tensor_scalar(out=b3m[:], in0=b3b[:],
                                        scalar1=blob_t[:, 1:2], scalar2=None,
                                        op0=AO.mult)
                nc.vector.tensor_tensor(out=agg_sb[:], in0=agg_sb[:],
                                        in1=b3m[:], op=AO.add)
                trp = ps_s.tile([P, P], f32, tag="ps_small")
                nc.tensor.transpose(out=trp[:], in_=agg_sb[:],
                                    identity=identity[:])
                agg16 = sbh.tile([P, P], f16, tag="agg16")
                nc.scalar.activation(out=agg16[:], in_=trp[:], func=Copy)

                n1p = ps_s.tile([P, P], f32, tag="ps_small")
                mm(out=n1p[:], lhsT=W[f"pn_W1h_{s}"][:], rhs=h16[:],
                   start=True, stop=False)
                mm(out=n1p[:], lhsT=W[f"pn_W1a_{s}"][:], rhs=agg16[:],
                   start=False, stop=True)
                n1 = sbn.tile([P, P], f16, tag="n1")
                nc.vector.tensor_scalar(out=n1[:], in0=n1p[:],
                                        scalar1=W[f"pn_b1_{s}"][:, :1],
                                        scalar2=0.0, op0=AO.add, op1=AO.max)
                n2p = ps_s.tile([P, P], f32, tag="ps_small")
                mm(out=n2p[:], lhsT=W[f"pn_W2_{s}"][:], rhs=n1[:],
                   start=True, stop=True)
                n2 = sbn.tile([P, P], f16, tag="n2")
                nc.scalar.activation(out=n2[:], in_=n2p[:], func=Relu,
                                     bias=W[f"pn_b2_{s}"][:, :1])
                n3p = ps_s.tile([P, P], f32, tag="ps_small")
                mm(out=n3p[:], lhsT=W[f"pn_W3_{s}"][:], rhs=n2[:],
                   start=True, stop=True)
                delta = sbn.tile([P, P], f32, tag="delta")
                nc.vector.tensor_scalar(out=delta[:], in0=n3p[:],
                                        scalar1=W[f"pn_b3_{s}"][:, :1],
                                        scalar2=None, op0=AO.add)
                hnew = sbh.tile([P, P], f32, tag="hnew")
                nc.vector.tensor_tensor(out=hnew[:], in0=hfm_t[:],
                                        in1=delta[:], op=AO.add)
                nc.sync.dma_start(out=h_fm[wr][ts(b, P), :], in_=hnew[:])
                if s < S - 1:
                    trh = ps_s.tile([P, P], f32, tag="ps_small")
                    nc.tensor.transpose(out=trh[:], in_=hnew[:],
                                        identity=identity[:])
                    hn16 = sbh.tile([P, P], f16, tag="hn16")
                    nc.scalar.activation(out=hn16[:], in_=trh[:], func=Copy)
                    nc.scalar.dma_start(out=h_own[ts(b, P), :], in_=hn16[:])
            if s < S - 1:
                nc.gpsimd.collective_compute(
                    "AllGather", mybir.AluOpType.bypass,
                    replica_groups=[list(range(n_cores))],
                    ins=[h_own[:, :]], outs=[h_tbl[wr][:, :]])

        # ---- decoder ----
        with tc.For_i(0, BPC, 1, staggered_reset=True) as b:
            hfm_t = sbh.tile([P, P], f32, tag="dec_h")
            nc.sync.dma_start(out=hfm_t[:], in_=h_fm[S % 2][ts(b, P), :])
            h16 = sbn.tile([P, P], f16, tag="dec_h16")
            nc.scalar.activation(out=h16[:], in_=hfm_t[:], func=Copy)
            d1p = ps_s.tile([P, P], f32, tag="ps_small")
            mm(out=d1p[:], lhsT=W["de_W1"][:], rhs=h16[:], start=True,
               stop=True)
            d1 = sbn.tile([P, P], f16, tag="dec_d1")
            nc.scalar.activation(out=d1[:], in_=d1p[:], func=Relu,
                                 bias=W["de_b1"][:, :1])
            d2p = ps_s.tile([P, P], f32, tag="ps_small")
            mm(out=d2p[:], lhsT=W["de_W2"][:], rhs=d1[:], start=True,
               stop=True)
            d2 = sbn.tile([P, P], f16, tag="dec_d2")
            nc.vector.tensor_scalar(out=d2[:], in0=d2p[:],
                                    scalar1=W["de_b2"][:, :1], scalar2=0.0,
                                    op0=AO.add, op1=AO.max)
            dp = ps_s.tile([P, OD], f32, tag="ps_small")
            mm(out=dp[:], lhsT=d2[:], rhs=W["de_W3"][:], start=True,
               stop=False)
            mm(out=dp[:], lhsT=ones_row[:], rhs=W["de_b3"][:], start=False,
               stop=True)
            osb = sbn.tile([P, OD], f32, tag="osb")
            nc.vector.tensor_copy(osb[:], dp[:])
            nc.scalar.dma_start(out=out_d[ts(b, P), :], in_=osb[:])

    nc.finalize()
    return nc


def _ensure_ntff_hook():
    """Register the axon NTFF profiling hook if the image lacks
    antenv.axon_hooks (replicates trn_boot's ctypes wiring)."""
    import sys
    import types
    try:
        import antenv.axon_hooks  # noqa: F401
        return
    except ImportError:
        pass
    import contextlib
    import ctypes
    import antenv

    m = types.ModuleType("antenv.axon_hooks")
    state = {"hook": None, "tried": False}

    def set_axon_ntff_profile_hook(hook):
        state["hook"] = hook

    def _make_hook(so_path="/opt/axon/libaxon_pjrt.so"):
        lib = ctypes.CDLL(so_path)
        if not hasattr(lib, "axon_start_nrt_profile"):
            return None
        lib.axon_start_nrt_profile.argtypes = [
            ctypes.POINTER(ctypes.c_int64), ctypes.c_size_t]
        lib.axon_start_nrt_profile.restype = ctypes.c_int64
        lib.axon_stop_nrt_profile.argtypes = [ctypes.c_char_p]
        lib.axon_stop_nrt_profile.restype = ctypes.c_int64

        @contextlib.contextmanager
        def _hook(output_dir, device_ids):
            import jax
            jax.devices()
            if device_ids:
                ids = (ctypes.c_int64 * len(device_ids))(*device_ids)
                rc = lib.axon_start_nrt_profile(ids, len(device_ids))
            else:
                rc = lib.axon_start_nrt_profile(None, 0)
            if rc != 0:
                raise RuntimeError(f"axon_start_nrt_profile rc={rc}")
            try:
                yield
            finally:
                n = lib.axon_stop_nrt_profile(str(output_dir).encode())
                print(f"ntff profile: {n} file(s) written to {output_dir}")

        return _hook

    def get_axon_ntff_profile_hook():
        if state["hook"] is None and not state["tried"]:
            state["tried"] = True
            try:
                state["hook"] = _make_hook()
            except OSError:
                state["hook"] = None
        return state["hook"]

    m.set_axon_ntff_profile_hook = set_axon_ntff_profile_hook
    m.get_axon_ntff_profile_hook = get_axon_ntff_profile_hook
    sys.modules["antenv.axon_hooks"] = m
    antenv.axon_hooks = m


def kernel(**inputs):
    n_cores = 8
    params, in_maps = prep_host(inputs, n_cores)
    nc = build_program(params, debug=False)

    from concourse.bass_utils import run_bass_kernel_spmd
    import time
    trace = bool(int(os.environ.get("KERNEL_TRACE", "0")))
    if trace:
        try:
            _ensure_ntff_hook()
        except Exception:
            pass
    t0 = time.time()
    try:
        res = run_bass_kernel_spmd(nc, in_maps, list(range(n_cores)),
                                   trace=trace)
    except ModuleNotFoundError:
        res = run_bass_kernel_spmd(nc, in_maps, list(range(n_cores)),
                                   trace=False)
    LAST["wall_s"] = time.time() - t0
    LAST["exec_time_ns"] = getattr(res, "exec_time_ns", None)
    LAST["profile_json"] = getattr(res, "profile_json", None)
    LAST["params"] = params
    out = np.concatenate([r["out"] for r in res.results], axis=0)
    return np.ascontiguousarray(out[:params["N"]].astype(np.float32))


# revision 13
# speedup vs baseline: 1.1655x; 1.1655x over previous
"""Trainium2 Bass kernel: MeshGraphNet-style GNN message passing (v2).

Strategy (8 NeuronCores, SPMD, edges partitioned by dst block):
  - Sort edges by dst; nodes in 128-blocks; each core owns a contiguous
    range of 49 blocks and all edges targeting them.
  - Per-step node state h lives in DRAM twice: a per-core fp32
    feature-major copy (residual-precision master, ping-pong A/B) and a
    replicated fp16 node-major gather table (AllGathered each step).
  - h[src] is fetched with dma_gather(transpose=True): 256B fp16 rows,
    delivered ALREADY feature-major - no PE transposes.  The int16 index
    limit (32767 < N) is dodged by splitting each block's edge list into
    a low-src part (src < 32768) and a high-src part, each padded to a
    uniform per-block boundary so the For_i body stays static; the high
    gather reads a row-offset slice of the same table.
  - h[dst] never materializes: pre1 += A_nm @ O_T where A_nm = h@W1d per
    own node (one matmul/block) and O_T is the host-precomputed one-hot
    (fp16, loaded per block-step).
  - The edge encoder's last layer is folded into the per-step W1e
    weights, so only relu2 (penultimate activation) is stored.
  - scatter-mean via one-hot matmul accumulation in PSUM (O_scat built
    on DVE from the dloc blob); mean + masked edge-bias as vector ops.
  - All matmuls in fp16 (full PE rate), PSUM accumulate fp32.
"""

import os
import numpy as np

P = 128
HALF = 32768  # int16 gather index limit

LAST = {}


def _ceil_div(a, b):
    return -(-a // b)


def _strips(T):
    out = []
    t0 = 0
    while t0 < T:
        k = min(4, T - t0)
        out.append((t0, k))
        t0 += k
    return out


def prep_host(inputs, n_cores=8):
    x = np.asarray(inputs["x"], np.float32)
    ea = np.asarray(inputs["edge_attr"], np.float32)
    ei = np.asarray(inputs["edge_index"], np.int32)
    N, NI = x.shape
    E, EI = ea.shape
    L = np.asarray(inputs["ne_W1"]).shape[1]
    OD = np.asarray(inputs["de_W3"]).shape[1]
    S = np.asarray(inputs["pe_W1"]).shape[0]

    NB = _ceil_div(N, P)
    NB = _ceil_div(NB, n_cores) * n_cores
    BPC = NB // n_cores
    N_pad = NB * P

    src = ei[0].astype(np.int64)
    dst = ei[1].astype(np.int64)
    perm = np.argsort(dst, kind="stable")
    src_s = src[perm].astype(np.int32)
    dst_s = dst[perm].astype(np.int32)
    ea_s = ea[perm]

    deg = np.bincount(dst, minlength=N_pad).astype(np.float32)
    inv_deg = (1.0 / np.maximum(deg, 1.0)).astype(np.float32)
    mask = (deg > 0).astype(np.float32)

    block_start = np.searchsorted(dst_s, np.arange(0, N_pad + 1, P))

    # split each block's edges into low-src / high-src parts
    lo_idx, hi_idx = [], []
    for b in range(NB):
        s0, s1 = int(block_start[b]), int(block_start[b + 1])
        sl = src_s[s0:s1]
        lo_idx.append(np.nonzero(sl < HALF)[0] + s0)
        hi_idx.append(np.nonzero(sl >= HALF)[0] + s0)
    KLo = max(P, max(_ceil_div(len(v), P) for v in lo_idx) * P)
    KHi = max(P, max(_ceil_div(len(v), P) for v in hi_idx) * P)
    E_blk = KLo + KHi
    T = E_blk // P
    C = 2 + T  # blob cols: inv_deg, mask, dloc[p, t]

    blob = np.zeros((NB, P, C), np.float32)
    blob[:, :, 0] = inv_deg.reshape(NB, P)
    blob[:, :, 1] = mask.reshape(NB, P)
    idx16 = np.zeros((NB, P, E_blk // 16), np.int16)
    ot = np.zeros((NB, P, E_blk), np.float16)
    ea_pack = np.zeros((NB, E_blk, EI), np.float32)
    for b in range(NB):
        li, hi = lo_idx[b], hi_idx[b]
        nl, nh = len(li), len(hi)
        dloc = np.full(E_blk, -1.0, np.float32)
        dloc[:nl] = (dst_s[li] - b * P).astype(np.float32)
        dloc[KLo:KLo + nh] = (dst_s[hi] - b * P).astype(np.float32)
        blob[b, :, 2:] = dloc.reshape(T, P).T
        gidx = np.zeros(E_blk, np.int64)
        gidx[:nl] = src_s[li]
        gidx[KLo:KLo + nh] = src_s[hi] - HALF
        w = gidx.reshape(-1, 16).astype(np.int16)  # [E_blk//16, 16]
        idx16[b] = np.tile(w.T, (P // 16, 1))  # [P, E_blk//16]
        ecols = np.arange(E_blk)
        valid = dloc >= 0
        ot[b, :, :] = (dloc[None, :] ==
                       np.arange(P, dtype=np.float32)[:, None]).astype(
                           np.float16)
        ot[b, :, ~valid] = 0
        ea_pack[b, :nl] = ea_s[li]
        ea_pack[b, KLo:KLo + nh] = ea_s[hi]

    x_fm = np.zeros((NI, N_pad), np.float16)
    x_fm[:, :N] = x.T.astype(np.float16)

    params = dict(N=N, NI=NI, E=E, EI=EI, L=L, OD=OD, S=S,
                  NB=NB, BPC=BPC, N_pad=N_pad, T=T, E_blk=E_blk, C=C,
                  KLo=KLo, KHi=KHi, n_cores=n_cores)

    def f16(a):
        return np.ascontiguousarray(np.asarray(a, np.float32).astype(
            np.float16))

    def f32(a):
        return np.ascontiguousarray(np.asarray(a, np.float32))

    ee_W3 = np.asarray(inputs["ee_W3"], np.float32)
    ee_b3 = np.asarray(inputs["ee_b3"], np.float32)
    pe_W1 = np.asarray(inputs["pe_W1"], np.float32)  # [S, 3L, L]
    pe_b1 = np.asarray(inputs["pe_b1"], np.float32)  # [S, L]

    weights = {
        "ne_W1": f16(inputs["ne_W1"]), "ne_W2": f16(inputs["ne_W2"]),
        "ne_W3": f16(inputs["ne_W3"]),
        "ee_W1": f16(inputs["ee_W1"]), "ee_W2": f16(inputs["ee_W2"]),
        "de_W1": f16(inputs["de_W1"]), "de_W2": f16(inputs["de_W2"]),
        "de_W3": f16(inputs["de_W3"]),
        "ne_b1": f32(inputs["ne_b1"]).reshape(L, 1),
        "ne_b2": f32(inputs["ne_b2"]).reshape(L, 1),
        "ne_b3": f32(inputs["ne_b3"]).reshape(L, 1),
        "ee_b1": f32(inputs["ee_b1"]).reshape(L, 1),
        "ee_b2": f32(inputs["ee_b2"]).reshape(L, 1),
        "de_b1": f32(inputs["de_b1"]).reshape(L, 1),
        "de_b2": f32(inputs["de_b2"]).reshape(L, 1),
        "de_b3": f16(np.asarray(inputs["de_b3"], np.float32).reshape(1, OD)),
        "pe_W1d": f16(pe_W1[:, 0:L, :].reshape(S * L, L)),
        "pe_W1s": f16(pe_W1[:, L:2 * L, :].reshape(S * L, L)),
        "pe_W1e": f16(np.einsum("kl,slm->skm", ee_W3,
                                pe_W1[:, 2 * L:3 * L, :]).reshape(S * L, L)),
        "pe_W2": f16(inputs["pe_W2"]).reshape(S * L, L),
        "pe_W3": f16(inputs["pe_W3"]).reshape(S * L, L),
        "pn_W1h": f16(np.asarray(inputs["pn_W1"],
                                 np.float32)[:, 0:L, :].reshape(S * L, L)),
        "pn_W1a": f16(np.asarray(inputs["pn_W1"],
                                 np.float32)[:, L:2 * L, :].reshape(S * L, L)),
        "pn_W2": f16(inputs["pn_W2"]).reshape(S * L, L),
        "pn_W3": f16(inputs["pn_W3"]).reshape(S * L, L),
        "pe_b1": f32(pe_b1 + np.einsum("l,slm->sm", ee_b3.reshape(L),
                                       pe_W1[:, 2 * L:3 * L, :])).reshape(
                                           S * L, 1),
        "pe_b2": f32(inputs["pe_b2"]).reshape(S * L, 1),
        "pe_b3": f16(inputs["pe_b3"]).reshape(S, L),
        "pn_b1": f32(inputs["pn_b1"]).reshape(S * L, 1),
        "pn_b2": f32(inputs["pn_b2"]).reshape(S * L, 1),
        "pn_b3": f32(inputs["pn_b3"]).reshape(S * L, 1),
    }

    in_maps = []
    for c in range(n_cores):
        b0, b1 = c * BPC, (c + 1) * BPC
        m = dict(weights)
        m["xfm"] = np.ascontiguousarray(x_fm[:, b0 * P:b1 * P])
        m["blob"] = np.ascontiguousarray(blob[b0:b1].reshape(BPC * P, C))
        m["idx"] = np.ascontiguousarray(
            idx16[b0:b1].reshape(BPC * P, E_blk // 16))
        m["ot"] = np.ascontiguousarray(ot[b0:b1].reshape(BPC * P, E_blk))
        m["eafm"] = np.ascontiguousarray(
            ea_pack[b0:b1].reshape(BPC * E_blk, EI).T.astype(np.float16))
        in_maps.append(m)
    return params, in_maps


def build_program(params, debug=False):
    import concourse.bass as bass
    import concourse.bacc as bacc
    import concourse.mybir as mybir
    import concourse.tile as tile
    from concourse.bass import ds, ts
    from concourse.masks import make_identity
    from contextlib import ExitStack

    f32 = mybir.dt.float32
    f16 = mybir.dt.float16
    i16 = mybir.dt.int16
    Relu = mybir.ActivationFunctionType.Relu
    Copy = mybir.ActivationFunctionType.Copy
    AO = mybir.AluOpType

    NI, EI, L, OD, S = (params[k] for k in ("NI", "EI", "L", "OD", "S"))
    BPC, N_pad, T, E_blk, C = (params[k] for k in
                               ("BPC", "N_pad", "T", "E_blk", "C"))
    KLo, KHi = params["KLo"], params["KHi"]
    n_cores = params["n_cores"]
    strips = _strips(T)

    nc = bacc.Bacc(None, target_bir_lowering=False, debug=debug,
                   num_swdge_queues=1,
                   dynamic_dma_scratch_size=32768)

    def par(name, shape, dtype=f32, out=False):
        return nc.declare_dram_parameter(name, list(shape), dtype, isOutput=out)

    xfm_d = par("xfm", [NI, BPC * P], f16)
    eafm_d = par("eafm", [EI, BPC * E_blk], f16)
    blob_d = par("blob", [BPC * P, C])
    idx_d = par("idx", [BPC * P, E_blk // 16], i16)
    ot_d = par("ot", [BPC * P, E_blk], f16)

    w_d = {}
    for nm, shp, dt in [
        ("ne_W1", [NI, L], f16), ("ne_W2", [L, L], f16), ("ne_W3", [L, L], f16),
        ("ee_W1", [EI, L], f16), ("ee_W2", [L, L], f16),
        ("de_W1", [L, L], f16), ("de_W2", [L, L], f16), ("de_W3", [L, OD], f16),
        ("ne_b1", [L, 1], f32), ("ne_b2", [L, 1], f32), ("ne_b3", [L, 1], f32),
        ("ee_b1", [L, 1], f32), ("ee_b2", [L, 1], f32),
        ("de_b1", [L, 1], f32), ("de_b2", [L, 1], f32), ("de_b3", [1, OD], f16),
        ("pe_W1d", [S * L, L], f16), ("pe_W1s", [S * L, L], f16),
        ("pe_W1e", [S * L, L], f16), ("pe_W2", [S * L, L], f16),
        ("pe_W3", [S * L, L], f16),
        ("pn_W1h", [S * L, L], f16), ("pn_W1a", [S * L, L], f16),
        ("pn_W2", [S * L, L], f16), ("pn_W3", [S * L, L], f16),
        ("pe_b1", [S * L, 1], f32), ("pe_b2", [S * L, 1], f32),
        ("pe_b3", [S, L], f16),
        ("pn_b1", [S * L, 1], f32), ("pn_b2", [S * L, 1], f32),
        ("pn_b3", [S * L, 1], f32),
    ]:
        w_d[nm] = par(nm, shp, dt)

    out_d = par("out", [BPC * P, OD], out=True)

    # fp16 node-major gather tables (ping-pong), fp32 fm h master (ping-pong)
    h_own = nc.dram_tensor("h_own", [BPC * P, L], f16)
    h_tblA = nc.dram_tensor("h_tblA", [N_pad, L], f16, addr_space="Shared")
    h_tblB = nc.dram_tensor("h_tblB", [N_pad, L], f16, addr_space="Shared")
    h_tbl = [h_tblA, h_tblB]
    h_fmA = nc.dram_tensor("h_fmA", [BPC * P, L], f32)
    h_fmB = nc.dram_tensor("h_fmB", [BPC * P, L], f32)
    h_fm = [h_fmA, h_fmB]
    relu2_d = nc.dram_tensor("relu2", [BPC * P, E_blk], f16)

    with tile.TileContext(nc) as tc, ExitStack() as ctx:
        wp = ctx.enter_context(tc.tile_pool(name="wp", bufs=1))
        sbh = ctx.enter_context(tc.tile_pool(name="sbh", bufs=3))
        sbg = ctx.enter_context(tc.tile_pool(name="sbg", bufs=6))
        sbe = ctx.enter_context(tc.tile_pool(name="sbe", bufs=2))
        sbo = ctx.enter_context(tc.tile_pool(name="sbo", bufs=5))
        sbr = ctx.enter_context(tc.tile_pool(name="sbr", bufs=5))
        sbm = ctx.enter_context(tc.tile_pool(name="sbm", bufs=6))
        sba = ctx.enter_context(tc.tile_pool(name="sba", bufs=3))
        sbs = ctx.enter_context(tc.tile_pool(name="sbs", bufs=2))
        sbn = ctx.enter_context(tc.tile_pool(name="sbn", bufs=3))
        ps_b = ctx.enter_context(tc.tile_pool(name="ps_b", bufs=3,
                                              space="PSUM"))
        ps_a = ctx.enter_context(tc.tile_pool(name="ps_a", bufs=2,
                                              space="PSUM"))
        ps_s = ctx.enter_context(tc.tile_pool(name="ps_s", bufs=3,
                                              space="PSUM"))

        identity = wp.tile([P, P], f32, tag="identity")
        make_identity(nc, identity[:])
        iota_i = wp.tile([P, P], mybir.dt.int32, tag="iota_i")
        nc.gpsimd.iota(iota_i[:], pattern=[[1, P]], base=0,
                       channel_multiplier=0)
        iota_f = wp.tile([P, P], f32, tag="iota_f")
        nc.vector.tensor_copy(iota_f[:], iota_i[:])
        ones_row = wp.tile([1, P], f16, tag="ones_row")
        nc.vector.memset(ones_row[:], 1.0)

        W = {}

        def load(nm, dram_ap, shape, tag, dt=f16):
            t = wp.tile(list(shape), dt, tag=tag)
            nc.sync.dma_start(out=t[:], in_=dram_ap)
            W[nm] = t
            return t

        for nm, shp in [("ne_W1", [NI, L]), ("ne_W2", [L, L]),
                        ("ne_W3", [L, L]), ("ee_W1", [EI, L]),
                        ("ee_W2", [L, L]), ("de_W1", [L, L]),
                        ("de_W2", [L, L]), ("de_W3", [L, OD])]:
            load(nm, w_d[nm][:, :], shp, nm)
        for nm in ("ne_b1", "ne_b2", "ne_b3", "ee_b1", "ee_b2",
                   "de_b1", "de_b2"):
            load(nm, w_d[nm][:, :], [L, 1], nm, f32)
        load("de_b3", w_d["de_b3"][:, :], [1, OD], "de_b3", f16)
        for s in range(S):
            for nm in ("pe_W1d", "pe_W1s", "pe_W1e", "pe_W2", "pe_W3",
                       "pn_W1h", "pn_W1a", "pn_W2", "pn_W3"):
                load(f"{nm}_{s}", w_d[nm][s * L:(s + 1) * L, :], [L, L],
                     f"{nm}_{s}")
            for nm in ("pe_b1", "pe_b2", "pn_b1", "pn_b2", "pn_b3"):
                load(f"{nm}_{s}", w_d[nm][s * L:(s + 1) * L, :], [L, 1],
                     f"{nm}_{s}", f32)
            load(f"pe_b3_{s}", w_d["pe_b3"][s:s + 1, :], [1, L],
                 f"pe_b3_{s}")

        mm = nc.tensor.matmul

        # ---- node encoder: h0 for own nodes -> h_fmA (f32) + table ----
        with tc.For_i(0, BPC, 1, staggered_reset=True) as b:
            x_t = sbn.tile([NI, P], f16, tag="x_t")
            nc.sync.dma_start(out=x_t[:], in_=xfm_d[:, ts(b, P)])
            p1 = ps_s.tile([P, P], f32, tag="ps_small")
            mm(out=p1[:], lhsT=W["ne_W1"][:], rhs=x_t[:], start=True,
               stop=True)
            a1 = sbn.tile([P, P], f16, tag="ne_a1")
            nc.scalar.activation(out=a1[:], in_=p1[:], func=Relu,
                                 bias=W["ne_b1"][:, :1])
            p2 = ps_s.tile([P, P], f32, tag="ps_small")
            mm(out=p2[:], lhsT=W["ne_W2"][:], rhs=a1[:], start=True,
               stop=True)
            a2 = sbn.tile([P, P], f16, tag="ne_a2")
            nc.vector.tensor_scalar(out=a2[:], in0=p2[:],
                                    scalar1=W["ne_b2"][:, :1], scalar2=0.0,
                                    op0=AO.add, op1=AO.max)
            p3 = ps_s.tile([P, P], f32, tag="ps_small")
            mm(out=p3[:], lhsT=W["ne_W3"][:], rhs=a2[:], start=True,
               stop=True)
            h0 = sbn.tile([P, P], f32, tag="ne_h0")
            nc.vector.tensor_scalar(out=h0[:], in0=p3[:],
                                    scalar1=W["ne_b3"][:, :1], scalar2=None,
                                    op0=AO.add)
            nc.sync.dma_start(out=h_fmA[ts(b, P), :], in_=h0[:])
            trp = ps_s.tile([P, P], f32, tag="ps_small")
            nc.tensor.transpose(out=trp[:], in_=h0[:], identity=identity[:])
            hn16 = sbn.tile([P, P], f16, tag="ne_hn16")
            nc.scalar.activation(out=hn16[:], in_=trp[:], func=Copy)
            nc.scalar.dma_start(out=h_own[ts(b, P), :], in_=hn16[:])
        nc.gpsimd.collective_compute(
            "AllGather", mybir.AluOpType.bypass,
            replica_groups=[list(range(n_cores))],
            ins=[h_own[:, :]], outs=[h_tblA[:, :]])

        # ---- edge encoder: relu2 for own edges -> relu2_d (fp16) ----
        with tc.For_i(0, BPC, 1, staggered_reset=True) as b:
            ea_t = sbe.tile([EI, E_blk], f16, tag="ea_t")
            nc.sync.dma_start(out=ea_t[:], in_=eafm_d[:, ts(b, E_blk)])
            r2_all = sbe.tile([P, E_blk], f16, tag="r2_all")
            for (t0, k) in strips:
                w = k * P
                cs = slice(t0 * P, t0 * P + w)
                p1 = ps_b.tile([P, 512], f32, tag="mm_big")
                mm(out=p1[:, :w], lhsT=W["ee_W1"][:], rhs=ea_t[:, cs],
                   start=True, stop=True)
                a1 = sba.tile([P, 512], f16, tag="ee_a1")
                nc.scalar.activation(out=a1[:, :w], in_=p1[:, :w], func=Relu,
                                     bias=W["ee_b1"][:, :1])
                p2 = ps_b.tile([P, 512], f32, tag="mm_big")
                mm(out=p2[:, :w], lhsT=W["ee_W2"][:], rhs=a1[:, :w],
                   start=True, stop=True)
                nc.vector.tensor_scalar(out=r2_all[:, cs], in0=p2[:, :w],
                                        scalar1=W["ee_b2"][:, :1], scalar2=0.0,
                                        op0=AO.add, op1=AO.max)
            nc.scalar.dma_start(out=relu2_d[ts(b, P), :], in_=r2_all[:])

        # ---- message passing steps ----
        for s in range(S):
            rd, wr = s % 2, (s + 1) % 2
            b3p = ps_s.tile([P, L], f32, tag="ps_small")
            mm(out=b3p[:], lhsT=ones_row[:], rhs=W[f"pe_b3_{s}"][:],
               start=True, stop=True)
            b3b = wp.tile([P, L], f32, tag=f"b3b_{s}")
            nc.vector.tensor_copy(b3b[:], b3p[:])

            with tc.For_i(0, BPC, 1, staggered_reset=True) as b:
                blob_t = sbm.tile([P, C], f32, tag="blob_t")
                nc.sync.dma_start(out=blob_t[:], in_=blob_d[ts(b, P), :])
                idx_t = sbm.tile([P, E_blk // 16], i16, tag="idx_t")
                nc.sync.dma_start(out=idx_t[:], in_=idx_d[ts(b, P), :])
                hfm_t = sbh.tile([P, P], f32, tag="hfm_t")
                nc.sync.dma_start(out=hfm_t[:], in_=h_fm[rd][ts(b, P), :])
                ot_t = sbo.tile([P, E_blk], f16, tag="ot_t")
                nc.scalar.dma_start(out=ot_t[:], in_=ot_d[ts(b, P), :])
                r2_t = sbr.tile([P, E_blk], f16, tag="r2_t")
                nc.scalar.dma_start(out=r2_t[:], in_=relu2_d[ts(b, P), :])

                hs3 = sbg.tile([P, 1, E_blk], f16, tag="hs3")
                CH = 512  # SWDGE descriptor-ring capacity limit per inst
                for c0 in range(0, KLo, CH):
                    w = min(CH, KLo - c0)
                    nc.gpsimd.dma_gather(
                        out_ap=hs3[:, :, c0:c0 + w], in_ap=h_tbl[rd][:, :],
                        idxs_ap=idx_t[:, c0 // 16:(c0 + w) // 16],
                        num_idxs=w, num_idxs_reg=w, elem_size=L,
                        transpose=True)
                for c0 in range(0, KHi, CH):
                    w = min(CH, KHi - c0)
                    nc.gpsimd.dma_gather(
                        out_ap=hs3[:, :, KLo + c0:KLo + c0 + w],
                        in_ap=h_tbl[rd][ds(HALF, N_pad - HALF), :],
                        idxs_ap=idx_t[:, (KLo + c0) // 16:(KLo + c0 + w) // 16],
                        num_idxs=w, num_idxs_reg=w, elem_size=L,
                        transpose=True)

                h16 = sbh.tile([P, P], f16, tag="h16")
                nc.scalar.activation(out=h16[:], in_=hfm_t[:], func=Copy)
                ap_nm = ps_s.tile([P, P], f32, tag="ps_small")
                mm(out=ap_nm[:], lhsT=h16[:], rhs=W[f"pe_W1d_{s}"][:],
                   start=True, stop=True)
                a16 = sbh.tile([P, P], f16, tag="a16")
                nc.scalar.activation(out=a16[:], in_=ap_nm[:], func=Copy)

                osc = sbs.tile([P, T * P], f16, tag="osc")
                for t in range(T):
                    nc.vector.tensor_tensor(
                        out=osc[:, t * P:(t + 1) * P],
                        in0=blob_t[:, 2 + t:3 + t].to_broadcast([P, P])[:],
                        in1=iota_f[:], op=AO.is_equal)

                aggp = ps_a.tile([P, L], f32, tag="aggp")
                for si, (t0, k) in enumerate(strips):
                    w = k * P
                    cs = slice(t0 * P, t0 * P + w)
                    pre1 = ps_b.tile([P, 512], f32, tag="mm_big")
                    mm(out=pre1[:, :w], lhsT=a16[:], rhs=ot_t[:, cs],
                       start=True, stop=False)
                    mm(out=pre1[:, :w], lhsT=W[f"pe_W1s_{s}"][:],
                       rhs=hs3[:, 0, cs], start=False, stop=False)
                    mm(out=pre1[:, :w], lhsT=W[f"pe_W1e_{s}"][:],
                       rhs=r2_t[:, cs], start=False, stop=True)
                    a1 = sba.tile([P, 512], f16, tag="pe_a1")
                    if si % 2 == 0:
                        nc.scalar.activation(out=a1[:, :w], in_=pre1[:, :w],
                                             func=Relu,
                                             bias=W[f"pe_b1_{s}"][:, :1])
                    else:
                        nc.vector.tensor_scalar(out=a1[:, :w], in0=pre1[:, :w],
                                                scalar1=W[f"pe_b1_{s}"][:, :1],
                                                scalar2=0.0, op0=AO.add,
                                                op1=AO.max)
                    a2p = ps_b.tile([P, 512], f32, tag="mm_big")
                    mm(out=a2p[:, :w], lhsT=W[f"pe_W2_{s}"][:], rhs=a1[:, :w],
                       start=True, stop=True)
                    a2 = sba.tile([P, 512], f16, tag="pe_a2")
                    if si % 2 == 1:
                        nc.scalar.activation(out=a2[:, :w], in_=a2p[:, :w],
                                             func=Relu,
                                             bias=W[f"pe_b2_{s}"][:, :1])
                    else:
                        nc.vector.tensor_scalar(out=a2[:, :w], in0=a2p[:, :w],
                                                scalar1=W[f"pe_b2_{s}"][:, :1],
                                                scalar2=0.0, op0=AO.add,
                                                op1=AO.max)
                    m_all = ps_b.tile([P, 512], f32, tag="mm_big")
                    for j in range(k):
                        mm(out=m_all[:, j * P:(j + 1) * P],
                           lhsT=a2[:, j * P:(j + 1) * P],
                           rhs=W[f"pe_W3_{s}"][:], start=True, stop=True)
                    m16 = sba.tile([P, 512], f16, tag="m16")
                    if si % 2 == 0:
                        nc.vector.tensor_copy(m16[:, :w], m_all[:, :w])
                    else:
                        nc.scalar.activation(out=m16[:, :w], in_=m_all[:, :w],
                                             func=Copy)
                    for j in range(k):
                        t = t0 + j
                        mm(out=aggp[:], lhsT=osc[:, t * P:(t + 1) * P],
                           rhs=m16[:, j * P:(j + 1) * P],
                           start=(t == 0), stop=(t == T - 1))

                agg_sb = sbh.tile([P, L], f32, tag="agg_sb")
                nc.vector.tensor_scalar(out=agg_sb[:], in0=aggp[:],
                                        scalar1=blob_t[:, 0:1], scalar2=None,
                                        op0=AO.mult)
                b3m = sbh.tile([P, L], f32, tag="b3m")
                nc.vector.tensor_scalar(out=b3m[:], in0=b3b[:],
                                        scalar1=blob_t[:, 1:2], scalar2=None,
                                        op0=AO.mult)
                nc.vector.tensor_tensor(out=agg_sb[:], in0=agg_sb[:],
                                        in1=b3m[:], op=AO.add)
                trp = ps_s.tile([P, P], f32, tag="ps_small")
                nc.tensor.transpose(out=trp[:], in_=agg_sb[:],
                                    identity=identity[:])
                agg16 = sbh.tile([P, P], f16, tag="agg16")
                nc.scalar.activation(out=agg16[:], in_=trp[:], func=Copy)

                n1p = ps_s.tile([P, P], f32, tag="ps_small")
                mm(out=n1p[:], lhsT=W[f"pn_W1h_{s}"][:], rhs=h16[:],
                   start=True, stop=False)
                mm(out=n1p[:], lhsT=W[f"pn_W1a_{s}"][:], rhs=agg16[:],
                   start=False, stop=True)
                n1 = sbn.tile([P, P], f16, tag="n1")
                nc.vector.tensor_scalar(out=n1[:], in0=n1p[:],
                                        scalar1=W[f"pn_b1_{s}"][:, :1],
                                        scalar2=0.0, op0=AO.add, op1=AO.max)
                n2p = ps_s.tile([P, P], f32, tag="ps_small")
                mm(out=n2p[:], lhsT=W[f"pn_W2_{s}"][:], rhs=n1[:],
                   start=True, stop=True)
                n2 = sbn.tile([P, P], f16, tag="n2")
                nc.scalar.activation(out=n2[:], in_=n2p[:], func=Relu,
                                     bias=W[f"pn_b2_{s}"][:, :1])
                n3p = ps_s.tile([P, P], f32, tag="ps_small")
                mm(out=n3p[:], lhsT=W[f"pn_W3_{s}"][:], rhs=n2[:],
                   start=True, stop=True)
                delta = sbn.tile([P, P], f32, tag="delta")
                nc.vector.tensor_scalar(out=delta[:], in0=n3p[:],
                                        scalar1=W[f"pn_b3_{s}"][:, :1],
                                        scalar2=None, op0=AO.add)
                hnew = sbh.tile([P, P], f32, tag="hnew")
                nc.vector.tensor_tensor(out=hnew[:], in0=hfm_t[:],
                                        in1=delta[:], op=AO.add)
                nc.sync.dma_start(out=h_fm[wr][ts(b, P), :], in_=hnew[:])
                if s < S - 1:
                    trh = ps_s.tile([P, P], f32, tag="ps_small")
                    nc.tensor.transpose(out=trh[:], in_=hnew[:],
                                        identity=identity[:])
                    hn16 = sbh.tile([P, P], f16, tag="hn16")
                    nc.scalar.activation(out=hn16[:], in_=trh[:], func=Copy)
                    nc.scalar.dma_start(out=h_own[ts(b, P), :], in_=hn16[:])
            if s < S - 1:
                nc.gpsimd.collective_compute(
                    "AllGather", mybir.AluOpType.bypass,
                    replica_groups=[list(range(n_cores))],
                    ins=[h_own[:, :]], outs=[h_tbl[wr][:, :]])

        # ---- decoder ----
        with tc.For_i(0, BPC, 1, staggered_reset=True) as b:
            hfm_t = sbh.tile([P, P], f32, tag="dec_h")
            nc.sync.dma_start(out=hfm_t[:], in_=h_fm[S % 2][ts(b, P), :])
            h16 = sbn.tile([P, P], f16, tag="dec_h16")
            nc.scalar.activation(out=h16[:], in_=hfm_t[:], func=Copy)
            d1p = ps_s.tile([P, P], f32, tag="ps_small")
            mm(out=d1p[:], lhsT=W["de_W1"][:], rhs=h16[:], start=True,
               stop=True)
            d1 = sbn.tile([P, P], f16, tag="dec_d1")
            nc.scalar.activation(out=d1[:], in_=d1p[:], func=Relu,
                                 bias=W["de_b1"][:, :1])
            d2p = ps_s.tile([P, P], f32, tag="ps_small")
            mm(out=d2p[:], lhsT=W["de_W2"][:], rhs=d1[:], start=True,
               stop=True)
            d2 = sbn.tile([P, P], f16, tag="dec_d2")
            nc.vector.tensor_scalar(out=d2[:], in0=d2p[:],
                                    scalar1=W["de_b2"][:, :1], scalar2=0.0,
                                    op0=AO.add, op1=AO.max)
            dp = ps_s.tile([P, OD], f32, tag="ps_small")
            mm(out=dp[:], lhsT=d2[:], rhs=W["de_W3"][:], start=True,
               stop=False)
            mm(out=dp[:], lhsT=ones_row[:], rhs=W["de_b3"][:], start=False,
               stop=True)
            osb = sbn.tile([P, OD], f32, tag="osb")
            nc.vector.tensor_copy(osb[:], dp[:])
            nc.scalar.dma_start(out=out_d[ts(b, P), :], in_=osb[:])

    nc.finalize()
    return nc


def _ensure_ntff_hook():
    """Register the axon NTFF profiling hook if the image lacks
    antenv.axon_hooks (replicates trn_boot's ctypes wiring)."""
    import sys
    import types
    try:
        import antenv.axon_hooks  # noqa: F401
        return
    except ImportError:
        pass
    import contextlib
    import ctypes
    import antenv

    m = types.ModuleType("antenv.axon_hooks")
    state = {"hook": None, "tried": False}

    def set_axon_ntff_profile_hook(hook):
        state["hook"] = hook

    def _make_hook(so_path="/opt/axon/libaxon_pjrt.so"):
        lib = ctypes.CDLL(so_path)
        if not hasattr(lib, "axon_start_nrt_profile"):
            return None
        lib.axon_start_nrt_profile.argtypes = [
            ctypes.POINTER(ctypes.c_int64), ctypes.c_size_t]
        lib.axon_start_nrt_profile.restype = ctypes.c_int64
        lib.axon_stop_nrt_profile.argtypes = [ctypes.c_char_p]
        lib.axon_stop_nrt_profile.restype = ctypes.c_int64

        @contextlib.contextmanager
        def _hook(output_dir, device_ids):
            import jax
            jax.devices()
            if device_ids:
                ids = (ctypes.c_int64 * len(device_ids))(*device_ids)
                rc = lib.axon_start_nrt_profile(ids, len(device_ids))
            else:
                rc = lib.axon_start_nrt_profile(None, 0)
            if rc != 0:
                raise RuntimeError(f"axon_start_nrt_profile rc={rc}")
            try:
                yield
            finally:
                n = lib.axon_stop_nrt_profile(str(output_dir).encode())
                print(f"ntff profile: {n} file(s) written to {output_dir}")

        return _hook

    def get_axon_ntff_profile_hook():
        if state["hook"] is None and not state["tried"]:
            state["tried"] = True
            try:
                state["hook"] = _make_hook()
            except OSError:
                state["hook"] = None
        return state["hook"]

    m.set_axon_ntff_profile_hook = set_axon_ntff_profile_hook
    m.get_axon_ntff_profile_hook = get_axon_ntff_profile_hook
    sys.modules["antenv.axon_hooks"] = m
    antenv.axon_hooks = m


def kernel(**inputs):
    n_cores = 8
    params, in_maps = prep_host(inputs, n_cores)
    nc = build_program(params, debug=False)

    from concourse.bass_utils import run_bass_kernel_spmd
    import time
    trace = bool(int(os.environ.get("KERNEL_TRACE", "0")))
    if trace:
        try:
            _ensure_ntff_hook()
        except Exception:
            pass
    t0 = time.time()
    try:
        res = run_bass_kernel_spmd(nc, in_maps, list(range(n_cores)),
                                   trace=trace)
    except ModuleNotFoundError:
        res = run_bass_kernel_spmd(nc, in_maps, list(range(n_cores)),
                                   trace=False)
    LAST["wall_s"] = time.time() - t0
    LAST["exec_time_ns"] = getattr(res, "exec_time_ns", None)
    LAST["profile_json"] = getattr(res, "profile_json", None)
    LAST["params"] = params
    out = np.concatenate([r["out"] for r in res.results], axis=0)
    return np.ascontiguousarray(out[:params["N"]].astype(np.float32))


# revision 14
# speedup vs baseline: 1.2308x; 1.0560x over previous
"""Trainium2 Bass kernel: MeshGraphNet-style GNN message passing (v2).

Strategy (8 NeuronCores, SPMD, edges partitioned by dst block):
  - Sort edges by dst; nodes in 128-blocks; each core owns a contiguous
    range of 49 blocks and all edges targeting them.
  - Per-step node state h lives in DRAM twice: a per-core fp32
    feature-major copy (residual-precision master, ping-pong A/B) and a
    replicated fp16 node-major gather table (AllGathered each step).
  - h[src] is fetched with dma_gather(transpose=True): 256B fp16 rows,
    delivered ALREADY feature-major - no PE transposes.  The int16 index
    limit (32767 < N) is dodged by splitting each block's edge list into
    a low-src part (src < 32768) and a high-src part, each padded to a
    uniform per-block boundary so the For_i body stays static; the high
    gather reads a row-offset slice of the same table.
  - h[dst] never materializes: pre1 += A_nm @ O_T where A_nm = h@W1d per
    own node (one matmul/block) and O_T is the host-precomputed one-hot
    (fp16, loaded per block-step).
  - The edge encoder's last layer is folded into the per-step W1e
    weights, so only relu2 (penultimate activation) is stored.
  - scatter-mean via one-hot matmul accumulation in PSUM (O_scat built
    on DVE from the dloc blob); mean + masked edge-bias as vector ops.
  - All matmuls in fp16 (full PE rate), PSUM accumulate fp32.
"""

import os
import numpy as np

P = 128
HALF = 32768  # int16 gather index limit

LAST = {}


def _ceil_div(a, b):
    return -(-a // b)


def _strips(T):
    out = []
    t0 = 0
    while t0 < T:
        k = min(4, T - t0)
        out.append((t0, k))
        t0 += k
    return out


def prep_host(inputs, n_cores=8):
    x = np.asarray(inputs["x"], np.float32)
    ea = np.asarray(inputs["edge_attr"], np.float32)
    ei = np.asarray(inputs["edge_index"], np.int32)
    N, NI = x.shape
    E, EI = ea.shape
    L = np.asarray(inputs["ne_W1"]).shape[1]
    OD = np.asarray(inputs["de_W3"]).shape[1]
    S = np.asarray(inputs["pe_W1"]).shape[0]

    NB = _ceil_div(N, P)
    NB = _ceil_div(NB, n_cores) * n_cores
    BPC = NB // n_cores
    N_pad = NB * P

    src = ei[0].astype(np.int64)
    dst = ei[1].astype(np.int64)
    perm = np.argsort(dst, kind="stable")
    src_s = src[perm].astype(np.int32)
    dst_s = dst[perm].astype(np.int32)
    ea_s = ea[perm]

    deg = np.bincount(dst, minlength=N_pad).astype(np.float32)
    inv_deg = (1.0 / np.maximum(deg, 1.0)).astype(np.float32)
    mask = (deg > 0).astype(np.float32)

    block_start = np.searchsorted(dst_s, np.arange(0, N_pad + 1, P))

    # split each block's edges into low-src / high-src parts
    lo_idx, hi_idx = [], []
    for b in range(NB):
        s0, s1 = int(block_start[b]), int(block_start[b + 1])
        sl = src_s[s0:s1]
        lo_idx.append(np.nonzero(sl < HALF)[0] + s0)
        hi_idx.append(np.nonzero(sl >= HALF)[0] + s0)
    KLo = max(P, max(_ceil_div(len(v), P) for v in lo_idx) * P)
    KHi = max(P, max(_ceil_div(len(v), P) for v in hi_idx) * P)
    E_blk = KLo + KHi
    T = E_blk // P
    C = 2 + T  # blob cols: inv_deg, mask, dloc[p, t]

    blob = np.zeros((NB, P, C), np.float32)
    blob[:, :, 0] = inv_deg.reshape(NB, P)
    blob[:, :, 1] = mask.reshape(NB, P)
    idx16 = np.zeros((NB, P, E_blk // 16), np.int16)
    ot = np.zeros((NB, P, E_blk), np.float16)
    ea_pack = np.zeros((NB, E_blk, EI), np.float32)
    for b in range(NB):
        li, hi = lo_idx[b], hi_idx[b]
        nl, nh = len(li), len(hi)
        dloc = np.full(E_blk, -1.0, np.float32)
        dloc[:nl] = (dst_s[li] - b * P).astype(np.float32)
        dloc[KLo:KLo + nh] = (dst_s[hi] - b * P).astype(np.float32)
        blob[b, :, 2:] = dloc.reshape(T, P).T
        gidx = np.zeros(E_blk, np.int64)
        gidx[:nl] = src_s[li]
        gidx[KLo:KLo + nh] = src_s[hi] - HALF
        w = gidx.reshape(-1, 16).astype(np.int16)  # [E_blk//16, 16]
        idx16[b] = np.tile(w.T, (P // 16, 1))  # [P, E_blk//16]
        ecols = np.arange(E_blk)
        valid = dloc >= 0
        ot[b, :, :] = (dloc[None, :] ==
                       np.arange(P, dtype=np.float32)[:, None]).astype(
                           np.float16)
        ot[b, :, ~valid] = 0
        ea_pack[b, :nl] = ea_s[li]
        ea_pack[b, KLo:KLo + nh] = ea_s[hi]

    x_fm = np.zeros((NI, N_pad), np.float16)
    x_fm[:, :N] = x.T.astype(np.float16)

    params = dict(N=N, NI=NI, E=E, EI=EI, L=L, OD=OD, S=S,
                  NB=NB, BPC=BPC, N_pad=N_pad, T=T, E_blk=E_blk, C=C,
                  KLo=KLo, KHi=KHi, n_cores=n_cores)

    def f16(a):
        return np.ascontiguousarray(np.asarray(a, np.float32).astype(
            np.float16))

    def f32(a):
        return np.ascontiguousarray(np.asarray(a, np.float32))

    ee_W3 = np.asarray(inputs["ee_W3"], np.float32)
    ee_b3 = np.asarray(inputs["ee_b3"], np.float32)
    pe_W1 = np.asarray(inputs["pe_W1"], np.float32)  # [S, 3L, L]
    pe_b1 = np.asarray(inputs["pe_b1"], np.float32)  # [S, L]

    weights = {
        "ne_W1": f16(inputs["ne_W1"]), "ne_W2": f16(inputs["ne_W2"]),
        "ne_W3": f16(inputs["ne_W3"]),
        "ee_W1": f16(inputs["ee_W1"]), "ee_W2": f16(inputs["ee_W2"]),
        "de_W1": f16(inputs["de_W1"]), "de_W2": f16(inputs["de_W2"]),
        "de_W3": f16(inputs["de_W3"]),
        "ne_b1": f32(inputs["ne_b1"]).reshape(L, 1),
        "ne_b2": f32(inputs["ne_b2"]).reshape(L, 1),
        "ne_b3": f32(inputs["ne_b3"]).reshape(L, 1),
        "ee_b1": f32(inputs["ee_b1"]).reshape(L, 1),
        "ee_b2": f32(inputs["ee_b2"]).reshape(L, 1),
        "de_b1": f32(inputs["de_b1"]).reshape(L, 1),
        "de_b2": f32(inputs["de_b2"]).reshape(L, 1),
        "de_b3": f16(np.asarray(inputs["de_b3"], np.float32).reshape(1, OD)),
        "pe_W1d": f16(pe_W1[:, 0:L, :].reshape(S * L, L)),
        "pe_W1s": f16(pe_W1[:, L:2 * L, :].reshape(S * L, L)),
        "pe_W1e": f16(np.einsum("kl,slm->skm", ee_W3,
                                pe_W1[:, 2 * L:3 * L, :]).reshape(S * L, L)),
        "pe_W2": f16(inputs["pe_W2"]).reshape(S * L, L),
        "pe_W3": f16(inputs["pe_W3"]).reshape(S * L, L),
        "pn_W1h": f16(np.asarray(inputs["pn_W1"],
                                 np.float32)[:, 0:L, :].reshape(S * L, L)),
        "pn_W1a": f16(np.asarray(inputs["pn_W1"],
                                 np.float32)[:, L:2 * L, :].reshape(S * L, L)),
        "pn_W2": f16(inputs["pn_W2"]).reshape(S * L, L),
        "pn_W3": f16(inputs["pn_W3"]).reshape(S * L, L),
        "pe_b1": f32(pe_b1 + np.einsum("l,slm->sm", ee_b3.reshape(L),
                                       pe_W1[:, 2 * L:3 * L, :])).reshape(
                                           S * L, 1),
        "pe_b2": f32(inputs["pe_b2"]).reshape(S * L, 1),
        "pe_b3": f16(inputs["pe_b3"]).reshape(S, L),
        "pn_b1": f32(inputs["pn_b1"]).reshape(S * L, 1),
        "pn_b2": f32(inputs["pn_b2"]).reshape(S * L, 1),
        "pn_b3": f32(inputs["pn_b3"]).reshape(S * L, 1),
    }

    in_maps = []
    for c in range(n_cores):
        b0, b1 = c * BPC, (c + 1) * BPC
        m = dict(weights)
        m["xfm"] = np.ascontiguousarray(x_fm[:, b0 * P:b1 * P])
        m["blob"] = np.ascontiguousarray(blob[b0:b1].reshape(BPC * P, C))
        m["idx"] = np.ascontiguousarray(
            idx16[b0:b1].reshape(BPC * P, E_blk // 16))
        m["ot"] = np.ascontiguousarray(ot[b0:b1].reshape(BPC * P, E_blk))
        m["eafm"] = np.ascontiguousarray(
            ea_pack[b0:b1].reshape(BPC * E_blk, EI).T.astype(np.float16))
        in_maps.append(m)
    return params, in_maps


def build_program(params, debug=False):
    import concourse.bass as bass
    import concourse.bacc as bacc
    import concourse.mybir as mybir
    import concourse.tile as tile
    from concourse.bass import ds, ts
    from concourse.masks import make_identity
    from contextlib import ExitStack

    f32 = mybir.dt.float32
    f16 = mybir.dt.float16
    i16 = mybir.dt.int16
    Relu = mybir.ActivationFunctionType.Relu
    Copy = mybir.ActivationFunctionType.Copy
    AO = mybir.AluOpType

    NI, EI, L, OD, S = (params[k] for k in ("NI", "EI", "L", "OD", "S"))
    BPC, N_pad, T, E_blk, C = (params[k] for k in
                               ("BPC", "N_pad", "T", "E_blk", "C"))
    KLo, KHi = params["KLo"], params["KHi"]
    n_cores = params["n_cores"]
    strips = _strips(T)

    nc = bacc.Bacc(None, target_bir_lowering=False, debug=debug,
                   num_swdge_queues=1,
                   dynamic_dma_scratch_size=32768)

    def par(name, shape, dtype=f32, out=False):
        return nc.declare_dram_parameter(name, list(shape), dtype, isOutput=out)

    xfm_d = par("xfm", [NI, BPC * P], f16)
    eafm_d = par("eafm", [EI, BPC * E_blk], f16)
    blob_d = par("blob", [BPC * P, C])
    idx_d = par("idx", [BPC * P, E_blk // 16], i16)
    ot_d = par("ot", [BPC * P, E_blk], f16)

    w_d = {}
    for nm, shp, dt in [
        ("ne_W1", [NI, L], f16), ("ne_W2", [L, L], f16), ("ne_W3", [L, L], f16),
        ("ee_W1", [EI, L], f16), ("ee_W2", [L, L], f16),
        ("de_W1", [L, L], f16), ("de_W2", [L, L], f16), ("de_W3", [L, OD], f16),
        ("ne_b1", [L, 1], f32), ("ne_b2", [L, 1], f32), ("ne_b3", [L, 1], f32),
        ("ee_b1", [L, 1], f32), ("ee_b2", [L, 1], f32),
        ("de_b1", [L, 1], f32), ("de_b2", [L, 1], f32), ("de_b3", [1, OD], f16),
        ("pe_W1d", [S * L, L], f16), ("pe_W1s", [S * L, L], f16),
        ("pe_W1e", [S * L, L], f16), ("pe_W2", [S * L, L], f16),
        ("pe_W3", [S * L, L], f16),
        ("pn_W1h", [S * L, L], f16), ("pn_W1a", [S * L, L], f16),
        ("pn_W2", [S * L, L], f16), ("pn_W3", [S * L, L], f16),
        ("pe_b1", [S * L, 1], f32), ("pe_b2", [S * L, 1], f32),
        ("pe_b3", [S, L], f16),
        ("pn_b1", [S * L, 1], f32), ("pn_b2", [S * L, 1], f32),
        ("pn_b3", [S * L, 1], f32),
    ]:
        w_d[nm] = par(nm, shp, dt)

    out_d = par("out", [BPC * P, OD], out=True)

    # fp16 node-major gather tables (ping-pong), fp32 fm h master (ping-pong)
    h_own = nc.dram_tensor("h_own", [BPC * P, L], f16)
    h_tblA = nc.dram_tensor("h_tblA", [N_pad, L], f16, addr_space="Shared")
    h_tblB = nc.dram_tensor("h_tblB", [N_pad, L], f16, addr_space="Shared")
    h_tbl = [h_tblA, h_tblB]
    h_fmA = nc.dram_tensor("h_fmA", [BPC * P, L], f32)
    h_fmB = nc.dram_tensor("h_fmB", [BPC * P, L], f32)
    h_fm = [h_fmA, h_fmB]
    relu2_d = nc.dram_tensor("relu2", [BPC * P, E_blk], f16)

    with tile.TileContext(nc) as tc, ExitStack() as ctx:
        wp = ctx.enter_context(tc.tile_pool(name="wp", bufs=1))
        sbh = ctx.enter_context(tc.tile_pool(name="sbh", bufs=3))
        sbg = ctx.enter_context(tc.tile_pool(name="sbg", bufs=6))
        sbe = ctx.enter_context(tc.tile_pool(name="sbe", bufs=2))
        sbo = ctx.enter_context(tc.tile_pool(name="sbo", bufs=5))
        sbr = ctx.enter_context(tc.tile_pool(name="sbr", bufs=5))
        sbm = ctx.enter_context(tc.tile_pool(name="sbm", bufs=6))
        sba = ctx.enter_context(tc.tile_pool(name="sba", bufs=3))
        sbs = ctx.enter_context(tc.tile_pool(name="sbs", bufs=2))
        sbn = ctx.enter_context(tc.tile_pool(name="sbn", bufs=3))
        ps_b = ctx.enter_context(tc.tile_pool(name="ps_b", bufs=3,
                                              space="PSUM"))
        ps_a = ctx.enter_context(tc.tile_pool(name="ps_a", bufs=2,
                                              space="PSUM"))
        ps_s = ctx.enter_context(tc.tile_pool(name="ps_s", bufs=3,
                                              space="PSUM"))

        identity = wp.tile([P, P], f32, tag="identity")
        make_identity(nc, identity[:])
        iota_i = wp.tile([P, P], mybir.dt.int32, tag="iota_i")
        nc.gpsimd.iota(iota_i[:], pattern=[[1, P]], base=0,
                       channel_multiplier=0)
        iota_f = wp.tile([P, P], f32, tag="iota_f")
        nc.vector.tensor_copy(iota_f[:], iota_i[:])
        ones_row = wp.tile([1, P], f16, tag="ones_row")
        nc.vector.memset(ones_row[:], 1.0)

        W = {}

        def load(nm, dram_ap, shape, tag, dt=f16):
            t = wp.tile(list(shape), dt, tag=tag)
            nc.sync.dma_start(out=t[:], in_=dram_ap)
            W[nm] = t
            return t

        for nm, shp in [("ne_W1", [NI, L]), ("ne_W2", [L, L]),
                        ("ne_W3", [L, L]), ("ee_W1", [EI, L]),
                        ("ee_W2", [L, L]), ("de_W1", [L, L]),
                        ("de_W2", [L, L]), ("de_W3", [L, OD])]:
            load(nm, w_d[nm][:, :], shp, nm)
        for nm in ("ne_b1", "ne_b2", "ne_b3", "ee_b1", "ee_b2",
                   "de_b1", "de_b2"):
            load(nm, w_d[nm][:, :], [L, 1], nm, f32)
        load("de_b3", w_d["de_b3"][:, :], [1, OD], "de_b3", f16)
        for s in range(S):
            for nm in ("pe_W1d", "pe_W1s", "pe_W1e", "pe_W2", "pe_W3",
                       "pn_W1h", "pn_W1a", "pn_W2", "pn_W3"):
                load(f"{nm}_{s}", w_d[nm][s * L:(s + 1) * L, :], [L, L],
                     f"{nm}_{s}")
            for nm in ("pe_b1", "pe_b2", "pn_b1", "pn_b2", "pn_b3"):
                load(f"{nm}_{s}", w_d[nm][s * L:(s + 1) * L, :], [L, 1],
                     f"{nm}_{s}", f32)
            load(f"pe_b3_{s}", w_d["pe_b3"][s:s + 1, :], [1, L],
                 f"pe_b3_{s}")

        mm = nc.tensor.matmul

        # ---- node encoder: h0 for own nodes -> h_fmA (f32) + table ----
        for b in range(BPC):
            x_t = sbn.tile([NI, P], f16, tag="x_t")
            nc.sync.dma_start(out=x_t[:], in_=xfm_d[:, ts(b, P)])
            p1 = ps_s.tile([P, P], f32, tag="ps_small")
            mm(out=p1[:], lhsT=W["ne_W1"][:], rhs=x_t[:], start=True,
               stop=True)
            a1 = sbn.tile([P, P], f16, tag="ne_a1")
            nc.scalar.activation(out=a1[:], in_=p1[:], func=Relu,
                                 bias=W["ne_b1"][:, :1])
            p2 = ps_s.tile([P, P], f32, tag="ps_small")
            mm(out=p2[:], lhsT=W["ne_W2"][:], rhs=a1[:], start=True,
               stop=True)
            a2 = sbn.tile([P, P], f16, tag="ne_a2")
            nc.vector.tensor_scalar(out=a2[:], in0=p2[:],
                                    scalar1=W["ne_b2"][:, :1], scalar2=0.0,
                                    op0=AO.add, op1=AO.max)
            p3 = ps_s.tile([P, P], f32, tag="ps_small")
            mm(out=p3[:], lhsT=W["ne_W3"][:], rhs=a2[:], start=True,
               stop=True)
            h0 = sbn.tile([P, P], f32, tag="ne_h0")
            nc.vector.tensor_scalar(out=h0[:], in0=p3[:],
                                    scalar1=W["ne_b3"][:, :1], scalar2=None,
                                    op0=AO.add)
            nc.sync.dma_start(out=h_fmA[ts(b, P), :], in_=h0[:])
            trp = ps_s.tile([P, P], f32, tag="ps_small")
            nc.tensor.transpose(out=trp[:], in_=h0[:], identity=identity[:])
            hn16 = sbn.tile([P, P], f16, tag="ne_hn16")
            nc.scalar.activation(out=hn16[:], in_=trp[:], func=Copy)
            nc.scalar.dma_start(out=h_own[ts(b, P), :], in_=hn16[:])
        nc.gpsimd.collective_compute(
            "AllGather", mybir.AluOpType.bypass,
            replica_groups=[list(range(n_cores))],
            ins=[h_own[:, :]], outs=[h_tblA[:, :]])

        # ---- edge encoder: relu2 for own edges -> relu2_d (fp16) ----
        for b in range(BPC):
            ea_t = sbe.tile([EI, E_blk], f16, tag="ea_t")
            nc.sync.dma_start(out=ea_t[:], in_=eafm_d[:, ts(b, E_blk)])
            r2_all = sbe.tile([P, E_blk], f16, tag="r2_all")
            for (t0, k) in strips:
                w = k * P
                cs = slice(t0 * P, t0 * P + w)
                p1 = ps_b.tile([P, 512], f32, tag="mm_big")
                mm(out=p1[:, :w], lhsT=W["ee_W1"][:], rhs=ea_t[:, cs],
                   start=True, stop=True)
                a1 = sba.tile([P, 512], f16, tag="ee_a1")
                nc.scalar.activation(out=a1[:, :w], in_=p1[:, :w], func=Relu,
                                     bias=W["ee_b1"][:, :1])
                p2 = ps_b.tile([P, 512], f32, tag="mm_big")
                mm(out=p2[:, :w], lhsT=W["ee_W2"][:], rhs=a1[:, :w],
                   start=True, stop=True)
                nc.vector.tensor_scalar(out=r2_all[:, cs], in0=p2[:, :w],
                                        scalar1=W["ee_b2"][:, :1], scalar2=0.0,
                                        op0=AO.add, op1=AO.max)
            nc.scalar.dma_start(out=relu2_d[ts(b, P), :], in_=r2_all[:])

        # ---- message passing steps ----
        for s in range(S):
            rd, wr = s % 2, (s + 1) % 2
            b3p = ps_s.tile([P, L], f32, tag="ps_small")
            mm(out=b3p[:], lhsT=ones_row[:], rhs=W[f"pe_b3_{s}"][:],
               start=True, stop=True)
            b3b = wp.tile([P, L], f32, tag=f"b3b_{s}")
            nc.vector.tensor_copy(b3b[:], b3p[:])

            for b in range(BPC):
                blob_t = sbm.tile([P, C], f32, tag="blob_t")
                nc.sync.dma_start(out=blob_t[:], in_=blob_d[ts(b, P), :])
                idx_t = sbm.tile([P, E_blk // 16], i16, tag="idx_t")
                nc.sync.dma_start(out=idx_t[:], in_=idx_d[ts(b, P), :])
                hfm_t = sbh.tile([P, P], f32, tag="hfm_t")
                nc.sync.dma_start(out=hfm_t[:], in_=h_fm[rd][ts(b, P), :])
                ot_t = sbo.tile([P, E_blk], f16, tag="ot_t")
                nc.scalar.dma_start(out=ot_t[:], in_=ot_d[ts(b, P), :])
                r2_t = sbr.tile([P, E_blk], f16, tag="r2_t")
                nc.scalar.dma_start(out=r2_t[:], in_=relu2_d[ts(b, P), :])

                hs3 = sbg.tile([P, 1, E_blk], f16, tag="hs3")
                CH = 512  # SWDGE descriptor-ring capacity limit per inst
                for c0 in range(0, KLo, CH):
                    w = min(CH, KLo - c0)
                    nc.gpsimd.dma_gather(
                        out_ap=hs3[:, :, c0:c0 + w], in_ap=h_tbl[rd][:, :],
                        idxs_ap=idx_t[:, c0 // 16:(c0 + w) // 16],
                        num_idxs=w, num_idxs_reg=w, elem_size=L,
                        transpose=True)
                for c0 in range(0, KHi, CH):
                    w = min(CH, KHi - c0)
                    nc.gpsimd.dma_gather(
                        out_ap=hs3[:, :, KLo + c0:KLo + c0 + w],
                        in_ap=h_tbl[rd][ds(HALF, N_pad - HALF), :],
                        idxs_ap=idx_t[:, (KLo + c0) // 16:(KLo + c0 + w) // 16],
                        num_idxs=w, num_idxs_reg=w, elem_size=L,
                        transpose=True)

                h16 = sbh.tile([P, P], f16, tag="h16")
                nc.scalar.activation(out=h16[:], in_=hfm_t[:], func=Copy)
                ap_nm = ps_s.tile([P, P], f32, tag="ps_small")
                mm(out=ap_nm[:], lhsT=h16[:], rhs=W[f"pe_W1d_{s}"][:],
                   start=True, stop=True)
                a16 = sbh.tile([P, P], f16, tag="a16")
                nc.scalar.activation(out=a16[:], in_=ap_nm[:], func=Copy)

                osc = sbs.tile([P, T * P], f16, tag="osc")
                for t in range(T):
                    nc.vector.tensor_tensor(
                        out=osc[:, t * P:(t + 1) * P],
                        in0=blob_t[:, 2 + t:3 + t].to_broadcast([P, P])[:],
                        in1=iota_f[:], op=AO.is_equal)

                aggp = ps_a.tile([P, L], f32, tag="aggp")
                for si, (t0, k) in enumerate(strips):
                    w = k * P
                    cs = slice(t0 * P, t0 * P + w)
                    pre1 = ps_b.tile([P, 512], f32, tag="mm_big")
                    mm(out=pre1[:, :w], lhsT=a16[:], rhs=ot_t[:, cs],
                       start=True, stop=False)
                    mm(out=pre1[:, :w], lhsT=W[f"pe_W1s_{s}"][:],
                       rhs=hs3[:, 0, cs], start=False, stop=False)
                    mm(out=pre1[:, :w], lhsT=W[f"pe_W1e_{s}"][:],
                       rhs=r2_t[:, cs], start=False, stop=True)
                    a1 = sba.tile([P, 512], f16, tag="pe_a1")
                    if si % 2 == 0:
                        nc.scalar.activation(out=a1[:, :w], in_=pre1[:, :w],
                                             func=Relu,
                                             bias=W[f"pe_b1_{s}"][:, :1])
                    else:
                        nc.vector.tensor_scalar(out=a1[:, :w], in0=pre1[:, :w],
                                                scalar1=W[f"pe_b1_{s}"][:, :1],
                                                scalar2=0.0, op0=AO.add,
                                                op1=AO.max)
                    a2p = ps_b.tile([P, 512], f32, tag="mm_big")
                    mm(out=a2p[:, :w], lhsT=W[f"pe_W2_{s}"][:], rhs=a1[:, :w],
                       start=True, stop=True)
                    a2 = sba.tile([P, 512], f16, tag="pe_a2")
                    if si % 2 == 1:
                        nc.scalar.activation(out=a2[:, :w], in_=a2p[:, :w],
                                             func=Relu,
                                             bias=W[f"pe_b2_{s}"][:, :1])
                    else:
                        nc.vector.tensor_scalar(out=a2[:, :w], in0=a2p[:, :w],
                                                scalar1=W[f"pe_b2_{s}"][:, :1],
                                                scalar2=0.0, op0=AO.add,
                                                op1=AO.max)
                    m_all = ps_b.tile([P, 512], f32, tag="mm_big")
                    for j in range(k):
                        mm(out=m_all[:, j * P:(j + 1) * P],
                           lhsT=a2[:, j * P:(j + 1) * P],
                           rhs=W[f"pe_W3_{s}"][:], start=True, stop=True)
                    m16 = sba.tile([P, 512], f16, tag="m16")
                    if si % 2 == 0:
                        nc.vector.tensor_copy(m16[:, :w], m_all[:, :w])
                    else:
                        nc.scalar.activation(out=m16[:, :w], in_=m_all[:, :w],
                                             func=Copy)
                    for j in range(k):
                        t = t0 + j
                        mm(out=aggp[:], lhsT=osc[:, t * P:(t + 1) * P],
                           rhs=m16[:, j * P:(j + 1) * P],
                           start=(t == 0), stop=(t == T - 1))

                agg_sb = sbh.tile([P, L], f32, tag="agg_sb")
                nc.vector.tensor_scalar(out=agg_sb[:], in0=aggp[:],
                                        scalar1=blob_t[:, 0:1], scalar2=None,
                                        op0=AO.mult)
                b3m = sbh.tile([P, L], f32, tag="b3m")
                nc.vector.tensor_scalar(out=b3m[:], in0=b3b[:],
                                        scalar1=blob_t[:, 1:2], scalar2=None,
                                        op0=AO.mult)
                nc.vector.tensor_tensor(out=agg_sb[:], in0=agg_sb[:],
                                        in1=b3m[:], op=AO.add)
                trp = ps_s.tile([P, P], f32, tag="ps_small")
                nc.tensor.transpose(out=trp[:], in_=agg_sb[:],
                                    identity=identity[:])
                agg16 = sbh.tile([P, P], f16, tag="agg16")
                nc.scalar.activation(out=agg16[:], in_=trp[:], func=Copy)

                n1p = ps_s.tile([P, P], f32, tag="ps_small")
                mm(out=n1p[:], lhsT=W[f"pn_W1h_{s}"][:], rhs=h16[:],
                   start=True, stop=False)
                mm(out=n1p[:], lhsT=W[f"pn_W1a_{s}"][:], rhs=agg16[:],
                   start=False, stop=True)
                n1 = sbn.tile([P, P], f16, tag="n1")
                nc.vector.tensor_scalar(out=n1[:], in0=n1p[:],
                                        scalar1=W[f"pn_b1_{s}"][:, :1],
                                        scalar2=0.0, op0=AO.add, op1=AO.max)
                n2p = ps_s.tile([P, P], f32, tag="ps_small")
                mm(out=n2p[:], lhsT=W[f"pn_W2_{s}"][:], rhs=n1[:],
                   start=True, stop=True)
                n2 = sbn.tile([P, P], f16, tag="n2")
                nc.scalar.activation(out=n2[:], in_=n2p[:], func=Relu,
                                     bias=W[f"pn_b2_{s}"][:, :1])
                n3p = ps_s.tile([P, P], f32, tag="ps_small")
                mm(out=n3p[:], lhsT=W[f"pn_W3_{s}"][:], rhs=n2[:],
                   start=True, stop=True)
                delta = sbn.tile([P, P], f32, tag="delta")
                nc.vector.tensor_scalar(out=delta[:], in0=n3p[:],
                                        scalar1=W[f"pn_b3_{s}"][:, :1],
                                        scalar2=None, op0=AO.add)
                hnew = sbh.tile([P, P], f32, tag="hnew")
                nc.vector.tensor_tensor(out=hnew[:], in0=hfm_t[:],
                                        in1=delta[:], op=AO.add)
                nc.sync.dma_start(out=h_fm[wr][ts(b, P), :], in_=hnew[:])
                if s < S - 1:
                    trh = ps_s.tile([P, P], f32, tag="ps_small")
                    nc.tensor.transpose(out=trh[:], in_=hnew[:],
                                        identity=identity[:])
                    hn16 = sbh.tile([P, P], f16, tag="hn16")
                    nc.scalar.activation(out=hn16[:], in_=trh[:], func=Copy)
                    nc.scalar.dma_start(out=h_own[ts(b, P), :], in_=hn16[:])
            if s < S - 1:
                nc.gpsimd.collective_compute(
                    "AllGather", mybir.AluOpType.bypass,
                    replica_groups=[list(range(n_cores))],
                    ins=[h_own[:, :]], outs=[h_tbl[wr][:, :]])

        # ---- decoder ----
        for b in range(BPC):
            hfm_t = sbh.tile([P, P], f32, tag="dec_h")
            nc.sync.dma_start(out=hfm_t[:], in_=h_fm[S % 2][ts(b, P), :])
            h16 = sbn.tile([P, P], f16, tag="dec_h16")
            nc.scalar.activation(out=h16[:], in_=hfm_t[:], func=Copy)
            d1p = ps_s.tile([P, P], f32, tag="ps_small")
            mm(out=d1p[:], lhsT=W["de_W1"][:], rhs=h16[:], start=True,
               stop=True)
            d1 = sbn.tile([P, P], f16, tag="dec_d1")
            nc.scalar.activation(out=d1[:], in_=d1p[:], func=Relu,
                                 bias=W["de_b1"][:, :1])
            d2p = ps_s.tile([P, P], f32, tag="ps_small")
            mm(out=d2p[:], lhsT=W["de_W2"][:], rhs=d1[:], start=True,
               stop=True)
            d2 = sbn.tile([P, P], f16, tag="dec_d2")
            nc.vector.tensor_scalar(out=d2[:], in0=d2p[:],
                                    scalar1=W["de_b2"][:, :1], scalar2=0.0,
                                    op0=AO.add, op1=AO.max)
            dp = ps_s.tile([P, OD], f32, tag="ps_small")
            mm(out=dp[:], lhsT=d2[:], rhs=W["de_W3"][:], start=True,
               stop=False)
            mm(out=dp[:], lhsT=ones_row[:], rhs=W["de_b3"][:], start=False,
               stop=True)
            osb = sbn.tile([P, OD], f32, tag="osb")
            nc.vector.tensor_copy(osb[:], dp[:])
            nc.scalar.dma_start(out=out_d[ts(b, P), :], in_=osb[:])

    nc.finalize()
    return nc


def _ensure_ntff_hook():
    """Register the axon NTFF profiling hook if the image lacks
    antenv.axon_hooks (replicates trn_boot's ctypes wiring)."""
    import sys
    import types
    try:
        import antenv.axon_hooks  # noqa: F401
        return
    except ImportError:
        pass
    import contextlib
    import ctypes
    import antenv

    m = types.ModuleType("antenv.axon_hooks")
    state = {"hook": None, "tried": False}

    def set_axon_ntff_profile_hook(hook):
        state["hook"] = hook

    def _make_hook(so_path="/opt/axon/libaxon_pjrt.so"):
        lib = ctypes.CDLL(so_path)
        if not hasattr(lib, "axon_start_nrt_profile"):
            return None
        lib.axon_start_nrt_profile.argtypes = [
            ctypes.POINTER(ctypes.c_int64), ctypes.c_size_t]
        lib.axon_start_nrt_profile.restype = ctypes.c_int64
        lib.axon_stop_nrt_profile.argtypes = [ctypes.c_char_p]
        lib.axon_stop_nrt_profile.restype = ctypes.c_int64

        @contextlib.contextmanager
        def _hook(output_dir, device_ids):
            import jax
            jax.devices()
            if device_ids:
                ids = (ctypes.c_int64 * len(device_ids))(*device_ids)
                rc = lib.axon_start_nrt_profile(ids, len(device_ids))
            else:
                rc = lib.axon_start_nrt_profile(None, 0)
            if rc != 0:
                raise RuntimeError(f"axon_start_nrt_profile rc={rc}")
            try:
                yield
            finally:
                n = lib.axon_stop_nrt_profile(str(output_dir).encode())
                print(f"ntff profile: {n} file(s) written to {output_dir}")

        return _hook

    def get_axon_ntff_profile_hook():
        if state["hook"] is None and not state["tried"]:
            state["tried"] = True
            try:
                state["hook"] = _make_hook()
            except OSError:
                state["hook"] = None
        return state["hook"]

    m.set_axon_ntff_profile_hook = set_axon_ntff_profile_hook
    m.get_axon_ntff_profile_hook = get_axon_ntff_profile_hook
    sys.modules["antenv.axon_hooks"] = m
    antenv.axon_hooks = m


def kernel(**inputs):
    n_cores = 8
    params, in_maps = prep_host(inputs, n_cores)
    nc = build_program(params, debug=False)

    from concourse.bass_utils import run_bass_kernel_spmd
    import time
    trace = bool(int(os.environ.get("KERNEL_TRACE", "0")))
    if trace:
        try:
            _ensure_ntff_hook()
        except Exception:
            pass
    t0 = time.time()
    try:
        res = run_bass_kernel_spmd(nc, in_maps, list(range(n_cores)),
                                   trace=trace)
    except ModuleNotFoundError:
        res = run_bass_kernel_spmd(nc, in_maps, list(range(n_cores)),
                                   trace=False)
    LAST["wall_s"] = time.time() - t0
    LAST["exec_time_ns"] = getattr(res, "exec_time_ns", None)
    LAST["profile_json"] = getattr(res, "profile_json", None)
    LAST["params"] = params
    out = np.concatenate([r["out"] for r in res.results], axis=0)
    return np.ascontiguousarray(out[:params["N"]].astype(np.float32))


# revision 15
# speedup vs baseline: 1.3741x; 1.1164x over previous
"""Trainium2 Bass kernel: MeshGraphNet-style GNN message passing (v2).

Strategy (8 NeuronCores, SPMD, edges partitioned by dst block):
  - Sort edges by dst; nodes in 128-blocks; each core owns a contiguous
    range of 49 blocks and all edges targeting them.
  - Per-step node state h lives in DRAM twice: a per-core fp32
    feature-major copy (residual-precision master, ping-pong A/B) and a
    replicated fp16 node-major gather table (AllGathered each step).
  - h[src] is fetched with dma_gather(transpose=True): 256B fp16 rows,
    delivered ALREADY feature-major - no PE transposes.  The int16 index
    limit (32767 < N) is dodged by splitting each block's edge list into
    a low-src part (src < 32768) and a high-src part, each padded to a
    uniform per-block boundary so the For_i body stays static; the high
    gather reads a row-offset slice of the same table.
  - h[dst] never materializes: pre1 += A_nm @ O_T where A_nm = h@W1d per
    own node (one matmul/block) and O_T is the host-precomputed one-hot
    (fp16, loaded per block-step).
  - The edge encoder's last layer is folded into the per-step W1e
    weights, so only relu2 (penultimate activation) is stored.
  - scatter-mean via one-hot matmul accumulation in PSUM (O_scat built
    on DVE from the dloc blob); mean + masked edge-bias as vector ops.
  - All matmuls in fp16 (full PE rate), PSUM accumulate fp32.
"""

import os
import numpy as np

P = 128
HALF = 32768  # int16 gather index limit

LAST = {}


def _ceil_div(a, b):
    return -(-a // b)


def _strips(T):
    out = []
    t0 = 0
    while t0 < T:
        k = min(4, T - t0)
        out.append((t0, k))
        t0 += k
    return out


def prep_host(inputs, n_cores=8):
    x = np.asarray(inputs["x"], np.float32)
    ea = np.asarray(inputs["edge_attr"], np.float32)
    ei = np.asarray(inputs["edge_index"], np.int32)
    N, NI = x.shape
    E, EI = ea.shape
    L = np.asarray(inputs["ne_W1"]).shape[1]
    OD = np.asarray(inputs["de_W3"]).shape[1]
    S = np.asarray(inputs["pe_W1"]).shape[0]

    NB = _ceil_div(N, P)
    NB = _ceil_div(NB, n_cores) * n_cores
    BPC = NB // n_cores
    N_pad = NB * P

    src = ei[0].astype(np.int64)
    dst = ei[1].astype(np.int64)
    perm = np.argsort(dst, kind="stable")
    src_s = src[perm].astype(np.int32)
    dst_s = dst[perm].astype(np.int32)
    ea_s = ea[perm]

    deg = np.bincount(dst, minlength=N_pad).astype(np.float32)
    inv_deg = (1.0 / np.maximum(deg, 1.0)).astype(np.float32)
    mask = (deg > 0).astype(np.float32)

    block_start = np.searchsorted(dst_s, np.arange(0, N_pad + 1, P))

    # split each block's edges into low-src / high-src parts
    lo_idx, hi_idx = [], []
    for b in range(NB):
        s0, s1 = int(block_start[b]), int(block_start[b + 1])
        sl = src_s[s0:s1]
        lo_idx.append(np.nonzero(sl < HALF)[0] + s0)
        hi_idx.append(np.nonzero(sl >= HALF)[0] + s0)
    KLo = max(P, max(_ceil_div(len(v), P) for v in lo_idx) * P)
    KHi = max(P, max(_ceil_div(len(v), P) for v in hi_idx) * P)
    E_blk = KLo + KHi
    T = E_blk // P
    C = 2 + T  # blob cols: inv_deg, mask, dloc[p, t]

    blob = np.zeros((NB, P, C), np.float32)
    blob[:, :, 0] = inv_deg.reshape(NB, P)
    blob[:, :, 1] = mask.reshape(NB, P)
    idx16 = np.zeros((NB, P, E_blk // 16), np.int16)
    ot = np.zeros((NB, P, E_blk), np.float16)
    ea_pack = np.zeros((NB, E_blk, EI), np.float32)
    for b in range(NB):
        li, hi = lo_idx[b], hi_idx[b]
        nl, nh = len(li), len(hi)
        dloc = np.full(E_blk, -1.0, np.float32)
        dloc[:nl] = (dst_s[li] - b * P).astype(np.float32)
        dloc[KLo:KLo + nh] = (dst_s[hi] - b * P).astype(np.float32)
        blob[b, :, 2:] = dloc.reshape(T, P).T
        gidx = np.zeros(E_blk, np.int64)
        gidx[:nl] = src_s[li]
        gidx[KLo:KLo + nh] = src_s[hi] - HALF
        w = gidx.reshape(-1, 16).astype(np.int16)  # [E_blk//16, 16]
        idx16[b] = np.tile(w.T, (P // 16, 1))  # [P, E_blk//16]
        ecols = np.arange(E_blk)
        valid = dloc >= 0
        ot[b, :, :] = (dloc[None, :] ==
                       np.arange(P, dtype=np.float32)[:, None]).astype(
                           np.float16)
        ot[b, :, ~valid] = 0
        ea_pack[b, :nl] = ea_s[li]
        ea_pack[b, KLo:KLo + nh] = ea_s[hi]

    x_fm = np.zeros((NI, N_pad), np.float16)
    x_fm[:, :N] = x.T.astype(np.float16)

    params = dict(N=N, NI=NI, E=E, EI=EI, L=L, OD=OD, S=S,
                  NB=NB, BPC=BPC, N_pad=N_pad, T=T, E_blk=E_blk, C=C,
                  KLo=KLo, KHi=KHi, n_cores=n_cores)

    def f16(a):
        return np.ascontiguousarray(np.asarray(a, np.float32).astype(
            np.float16))

    def f32(a):
        return np.ascontiguousarray(np.asarray(a, np.float32))

    ee_W3 = np.asarray(inputs["ee_W3"], np.float32)
    ee_b3 = np.asarray(inputs["ee_b3"], np.float32)
    pe_W1 = np.asarray(inputs["pe_W1"], np.float32)  # [S, 3L, L]
    pe_b1 = np.asarray(inputs["pe_b1"], np.float32)  # [S, L]

    weights = {
        "ne_W1": f16(inputs["ne_W1"]), "ne_W2": f16(inputs["ne_W2"]),
        "ne_W3": f16(inputs["ne_W3"]),
        "ee_W1": f16(inputs["ee_W1"]), "ee_W2": f16(inputs["ee_W2"]),
        "de_W1": f16(inputs["de_W1"]), "de_W2": f16(inputs["de_W2"]),
        "de_W3": f16(inputs["de_W3"]),
        "ne_b1": f32(inputs["ne_b1"]).reshape(L, 1),
        "ne_b2": f32(inputs["ne_b2"]).reshape(L, 1),
        "ne_b3": f32(inputs["ne_b3"]).reshape(L, 1),
        "ee_b1": f32(inputs["ee_b1"]).reshape(L, 1),
        "ee_b2": f32(inputs["ee_b2"]).reshape(L, 1),
        "de_b1": f32(inputs["de_b1"]).reshape(L, 1),
        "de_b2": f32(inputs["de_b2"]).reshape(L, 1),
        "de_b3": f16(np.asarray(inputs["de_b3"], np.float32).reshape(1, OD)),
        "pe_W1d": f16(pe_W1[:, 0:L, :].reshape(S * L, L)),
        "pe_W1s": f16(pe_W1[:, L:2 * L, :].reshape(S * L, L)),
        "pe_W1e": f16(np.einsum("kl,slm->skm", ee_W3,
                                pe_W1[:, 2 * L:3 * L, :]).reshape(S * L, L)),
        "pe_W2": f16(inputs["pe_W2"]).reshape(S * L, L),
        "pe_W3": f16(inputs["pe_W3"]).reshape(S * L, L),
        "pn_W1h": f16(np.asarray(inputs["pn_W1"],
                                 np.float32)[:, 0:L, :].reshape(S * L, L)),
        "pn_W1a": f16(np.asarray(inputs["pn_W1"],
                                 np.float32)[:, L:2 * L, :].reshape(S * L, L)),
        "pn_W2": f16(inputs["pn_W2"]).reshape(S * L, L),
        "pn_W3": f16(inputs["pn_W3"]).reshape(S * L, L),
        "pe_b1": f32(pe_b1 + np.einsum("l,slm->sm", ee_b3.reshape(L),
                                       pe_W1[:, 2 * L:3 * L, :])).reshape(
                                           S * L, 1),
        "pe_b2": f32(inputs["pe_b2"]).reshape(S * L, 1),
        "pe_b3": f16(inputs["pe_b3"]).reshape(S, L),
        "pn_b1": f32(inputs["pn_b1"]).reshape(S * L, 1),
        "pn_b2": f32(inputs["pn_b2"]).reshape(S * L, 1),
        "pn_b3": f32(inputs["pn_b3"]).reshape(S * L, 1),
    }

    in_maps = []
    for c in range(n_cores):
        b0, b1 = c * BPC, (c + 1) * BPC
        m = dict(weights)
        m["xfm"] = np.ascontiguousarray(x_fm[:, b0 * P:b1 * P])
        m["blob"] = np.ascontiguousarray(blob[b0:b1].reshape(BPC * P, C))
        m["idx"] = np.ascontiguousarray(
            idx16[b0:b1].reshape(BPC * P, E_blk // 16))
        m["ot"] = np.ascontiguousarray(ot[b0:b1].reshape(BPC * P, E_blk))
        m["eafm"] = np.ascontiguousarray(
            ea_pack[b0:b1].reshape(BPC * E_blk, EI).T.astype(np.float16))
        in_maps.append(m)
    return params, in_maps


def build_program(params, debug=False):
    import concourse.bass as bass
    import concourse.bacc as bacc
    import concourse.mybir as mybir
    import concourse.tile as tile
    from concourse.bass import ds, ts
    from concourse.masks import make_identity
    from contextlib import ExitStack

    f32 = mybir.dt.float32
    f16 = mybir.dt.float16
    i16 = mybir.dt.int16
    Relu = mybir.ActivationFunctionType.Relu
    Copy = mybir.ActivationFunctionType.Copy
    AO = mybir.AluOpType

    NI, EI, L, OD, S = (params[k] for k in ("NI", "EI", "L", "OD", "S"))
    BPC, N_pad, T, E_blk, C = (params[k] for k in
                               ("BPC", "N_pad", "T", "E_blk", "C"))
    KLo, KHi = params["KLo"], params["KHi"]
    n_cores = params["n_cores"]
    strips = _strips(T)

    nc = bacc.Bacc(None, target_bir_lowering=False, debug=debug,
                   num_swdge_queues=1,
                   dynamic_dma_scratch_size=32768)

    def par(name, shape, dtype=f32, out=False):
        return nc.declare_dram_parameter(name, list(shape), dtype, isOutput=out)

    xfm_d = par("xfm", [NI, BPC * P], f16)
    eafm_d = par("eafm", [EI, BPC * E_blk], f16)
    blob_d = par("blob", [BPC * P, C])
    idx_d = par("idx", [BPC * P, E_blk // 16], i16)
    ot_d = par("ot", [BPC * P, E_blk], f16)

    w_d = {}
    for nm, shp, dt in [
        ("ne_W1", [NI, L], f16), ("ne_W2", [L, L], f16), ("ne_W3", [L, L], f16),
        ("ee_W1", [EI, L], f16), ("ee_W2", [L, L], f16),
        ("de_W1", [L, L], f16), ("de_W2", [L, L], f16), ("de_W3", [L, OD], f16),
        ("ne_b1", [L, 1], f32), ("ne_b2", [L, 1], f32), ("ne_b3", [L, 1], f32),
        ("ee_b1", [L, 1], f32), ("ee_b2", [L, 1], f32),
        ("de_b1", [L, 1], f32), ("de_b2", [L, 1], f32), ("de_b3", [1, OD], f16),
        ("pe_W1d", [S * L, L], f16), ("pe_W1s", [S * L, L], f16),
        ("pe_W1e", [S * L, L], f16), ("pe_W2", [S * L, L], f16),
        ("pe_W3", [S * L, L], f16),
        ("pn_W1h", [S * L, L], f16), ("pn_W1a", [S * L, L], f16),
        ("pn_W2", [S * L, L], f16), ("pn_W3", [S * L, L], f16),
        ("pe_b1", [S * L, 1], f32), ("pe_b2", [S * L, 1], f32),
        ("pe_b3", [S, L], f16),
        ("pn_b1", [S * L, 1], f32), ("pn_b2", [S * L, 1], f32),
        ("pn_b3", [S * L, 1], f32),
    ]:
        w_d[nm] = par(nm, shp, dt)

    out_d = par("out", [BPC * P, OD], out=True)

    # fp16 node-major gather tables (ping-pong), fp32 fm h master (ping-pong)
    h_own = nc.dram_tensor("h_own", [BPC * P, L], f16)
    h_tblA = nc.dram_tensor("h_tblA", [N_pad, L], f16, addr_space="Shared")
    h_tblB = nc.dram_tensor("h_tblB", [N_pad, L], f16, addr_space="Shared")
    h_tbl = [h_tblA, h_tblB]
    h_fmA = nc.dram_tensor("h_fmA", [BPC * P, L], f32)
    h_fmB = nc.dram_tensor("h_fmB", [BPC * P, L], f32)
    h_fm = [h_fmA, h_fmB]
    relu2_d = nc.dram_tensor("relu2", [BPC * P, E_blk], f16)

    with tile.TileContext(nc) as tc, ExitStack() as ctx:
        wp = ctx.enter_context(tc.tile_pool(name="wp", bufs=1))
        sbh = ctx.enter_context(tc.tile_pool(name="sbh", bufs=3))
        sbg = ctx.enter_context(tc.tile_pool(name="sbg", bufs=6))
        sbe = ctx.enter_context(tc.tile_pool(name="sbe", bufs=2))
        sbo = ctx.enter_context(tc.tile_pool(name="sbo", bufs=5))
        sbr = ctx.enter_context(tc.tile_pool(name="sbr", bufs=5))
        sbm = ctx.enter_context(tc.tile_pool(name="sbm", bufs=6))
        sba = ctx.enter_context(tc.tile_pool(name="sba", bufs=3))
        sbs = ctx.enter_context(tc.tile_pool(name="sbs", bufs=2))
        sbn = ctx.enter_context(tc.tile_pool(name="sbn", bufs=3))
        ps_b = ctx.enter_context(tc.tile_pool(name="ps_b", bufs=3,
                                              space="PSUM"))
        ps_a = ctx.enter_context(tc.tile_pool(name="ps_a", bufs=2,
                                              space="PSUM"))
        ps_s = ctx.enter_context(tc.tile_pool(name="ps_s", bufs=3,
                                              space="PSUM"))

        identity = wp.tile([P, P], f32, tag="identity")
        make_identity(nc, identity[:])
        iota_i = wp.tile([P, P], mybir.dt.int32, tag="iota_i")
        nc.gpsimd.iota(iota_i[:], pattern=[[1, P]], base=0,
                       channel_multiplier=0)
        iota_f = wp.tile([P, P], f32, tag="iota_f")
        nc.vector.tensor_copy(iota_f[:], iota_i[:])
        ones_row = wp.tile([1, P], f16, tag="ones_row")
        nc.vector.memset(ones_row[:], 1.0)

        W = {}

        def load(nm, dram_ap, shape, tag, dt=f16):
            t = wp.tile(list(shape), dt, tag=tag)
            nc.sync.dma_start(out=t[:], in_=dram_ap)
            W[nm] = t
            return t

        for nm, shp in [("ne_W1", [NI, L]), ("ne_W2", [L, L]),
                        ("ne_W3", [L, L]), ("ee_W1", [EI, L]),
                        ("ee_W2", [L, L]), ("de_W1", [L, L]),
                        ("de_W2", [L, L]), ("de_W3", [L, OD])]:
            load(nm, w_d[nm][:, :], shp, nm)
        for nm in ("ne_b1", "ne_b2", "ne_b3", "ee_b1", "ee_b2",
                   "de_b1", "de_b2"):
            load(nm, w_d[nm][:, :], [L, 1], nm, f32)
        load("de_b3", w_d["de_b3"][:, :], [1, OD], "de_b3", f16)
        for s in range(S):
            for nm in ("pe_W1d", "pe_W1s", "pe_W1e", "pe_W2", "pe_W3",
                       "pn_W1h", "pn_W1a", "pn_W2", "pn_W3"):
                load(f"{nm}_{s}", w_d[nm][s * L:(s + 1) * L, :], [L, L],
                     f"{nm}_{s}")
            for nm in ("pe_b1", "pe_b2", "pn_b1", "pn_b2", "pn_b3"):
                load(f"{nm}_{s}", w_d[nm][s * L:(s + 1) * L, :], [L, 1],
                     f"{nm}_{s}", f32)
            load(f"pe_b3_{s}", w_d["pe_b3"][s:s + 1, :], [1, L],
                 f"pe_b3_{s}")

        mm = nc.tensor.matmul

        # ---- node encoder: h0 for own nodes -> h_fmA (f32) + table ----
        for b in range(BPC):
            x_t = sbn.tile([NI, P], f16, tag="x_t")
            nc.scalar.dma_start(out=x_t[:], in_=xfm_d[:, ts(b, P)])
            p1 = ps_s.tile([P, P], f32, tag="ps_small")
            mm(out=p1[:], lhsT=W["ne_W1"][:], rhs=x_t[:], start=True,
               stop=True)
            a1 = sbn.tile([P, P], f16, tag="ne_a1")
            nc.scalar.activation(out=a1[:], in_=p1[:], func=Relu,
                                 bias=W["ne_b1"][:, :1])
            p2 = ps_s.tile([P, P], f32, tag="ps_small")
            mm(out=p2[:], lhsT=W["ne_W2"][:], rhs=a1[:], start=True,
               stop=True)
            a2 = sbn.tile([P, P], f16, tag="ne_a2")
            nc.vector.tensor_scalar(out=a2[:], in0=p2[:],
                                    scalar1=W["ne_b2"][:, :1], scalar2=0.0,
                                    op0=AO.add, op1=AO.max)
            p3 = ps_s.tile([P, P], f32, tag="ps_small")
            mm(out=p3[:], lhsT=W["ne_W3"][:], rhs=a2[:], start=True,
               stop=True)
            h0 = sbn.tile([P, P], f32, tag="ne_h0")
            nc.vector.tensor_scalar(out=h0[:], in0=p3[:],
                                    scalar1=W["ne_b3"][:, :1], scalar2=None,
                                    op0=AO.add)
            nc.sync.dma_start(out=h_fmA[ts(b, P), :], in_=h0[:])
            trp = ps_s.tile([P, P], f32, tag="ps_small")
            nc.tensor.transpose(out=trp[:], in_=h0[:], identity=identity[:])
            hn16 = sbn.tile([P, P], f16, tag="ne_hn16")
            nc.vector.tensor_copy(hn16[:], trp[:])
            nc.sync.dma_start(out=h_own[ts(b, P), :], in_=hn16[:])
        nc.gpsimd.collective_compute(
            "AllGather", mybir.AluOpType.bypass,
            replica_groups=[list(range(n_cores))],
            ins=[h_own[:, :]], outs=[h_tblA[:, :]])

        # ---- edge encoder: relu2 for own edges -> relu2_d (fp16) ----
        for b in range(BPC):
            ea_t = sbe.tile([EI, E_blk], f16, tag="ea_t")
            nc.scalar.dma_start(out=ea_t[:], in_=eafm_d[:, ts(b, E_blk)])
            r2_all = sbe.tile([P, E_blk], f16, tag="r2_all")
            for (t0, k) in strips:
                w = k * P
                cs = slice(t0 * P, t0 * P + w)
                p1 = ps_b.tile([P, 512], f32, tag="mm_big")
                mm(out=p1[:, :w], lhsT=W["ee_W1"][:], rhs=ea_t[:, cs],
                   start=True, stop=True)
                a1 = sba.tile([P, 512], f16, tag="ee_a1")
                nc.scalar.activation(out=a1[:, :w], in_=p1[:, :w], func=Relu,
                                     bias=W["ee_b1"][:, :1])
                p2 = ps_b.tile([P, 512], f32, tag="mm_big")
                mm(out=p2[:, :w], lhsT=W["ee_W2"][:], rhs=a1[:, :w],
                   start=True, stop=True)
                nc.vector.tensor_scalar(out=r2_all[:, cs], in0=p2[:, :w],
                                        scalar1=W["ee_b2"][:, :1], scalar2=0.0,
                                        op0=AO.add, op1=AO.max)
            nc.sync.dma_start(out=relu2_d[ts(b, P), :], in_=r2_all[:])

        # ---- message passing steps ----
        for s in range(S):
            rd, wr = s % 2, (s + 1) % 2
            b3p = ps_s.tile([P, L], f32, tag="ps_small")
            mm(out=b3p[:], lhsT=ones_row[:], rhs=W[f"pe_b3_{s}"][:],
               start=True, stop=True)
            b3b = wp.tile([P, L], f32, tag=f"b3b_{s}")
            nc.vector.tensor_copy(b3b[:], b3p[:])

            for b in range(BPC):
                blob_t = sbm.tile([P, C], f32, tag="blob_t")
                nc.scalar.dma_start(out=blob_t[:], in_=blob_d[ts(b, P), :])
                idx_t = sbm.tile([P, E_blk // 16], i16, tag="idx_t")
                nc.scalar.dma_start(out=idx_t[:], in_=idx_d[ts(b, P), :])
                hfm_t = sbh.tile([P, P], f32, tag="hfm_t")
                nc.scalar.dma_start(out=hfm_t[:], in_=h_fm[rd][ts(b, P), :])
                ot_t = sbo.tile([P, E_blk], f16, tag="ot_t")
                nc.scalar.dma_start(out=ot_t[:], in_=ot_d[ts(b, P), :])
                r2_t = sbr.tile([P, E_blk], f16, tag="r2_t")
                nc.scalar.dma_start(out=r2_t[:], in_=relu2_d[ts(b, P), :])

                hs3 = sbg.tile([P, 1, E_blk], f16, tag="hs3")
                CH = 768  # SWDGE descriptor-ring capacity limit per inst
                for c0 in range(0, KLo, CH):
                    w = min(CH, KLo - c0)
                    nc.gpsimd.dma_gather(
                        out_ap=hs3[:, :, c0:c0 + w], in_ap=h_tbl[rd][:, :],
                        idxs_ap=idx_t[:, c0 // 16:(c0 + w) // 16],
                        num_idxs=w, num_idxs_reg=w, elem_size=L,
                        transpose=True)
                for c0 in range(0, KHi, CH):
                    w = min(CH, KHi - c0)
                    nc.gpsimd.dma_gather(
                        out_ap=hs3[:, :, KLo + c0:KLo + c0 + w],
                        in_ap=h_tbl[rd][ds(HALF, N_pad - HALF), :],
                        idxs_ap=idx_t[:, (KLo + c0) // 16:(KLo + c0 + w) // 16],
                        num_idxs=w, num_idxs_reg=w, elem_size=L,
                        transpose=True)

                h16 = sbh.tile([P, P], f16, tag="h16")
                nc.scalar.activation(out=h16[:], in_=hfm_t[:], func=Copy)
                ap_nm = ps_s.tile([P, P], f32, tag="ps_small")
                mm(out=ap_nm[:], lhsT=h16[:], rhs=W[f"pe_W1d_{s}"][:],
                   start=True, stop=True)
                a16 = sbh.tile([P, P], f16, tag="a16")
                nc.scalar.activation(out=a16[:], in_=ap_nm[:], func=Copy)

                osc = sbs.tile([P, T * P], f16, tag="osc")
                for t in range(T):
                    nc.vector.tensor_tensor(
                        out=osc[:, t * P:(t + 1) * P],
                        in0=blob_t[:, 2 + t:3 + t].to_broadcast([P, P])[:],
                        in1=iota_f[:], op=AO.is_equal)

                aggp = ps_a.tile([P, L], f32, tag="aggp")
                for si, (t0, k) in enumerate(strips):
                    w = k * P
                    cs = slice(t0 * P, t0 * P + w)
                    pre1 = ps_b.tile([P, 512], f32, tag="mm_big")
                    mm(out=pre1[:, :w], lhsT=a16[:], rhs=ot_t[:, cs],
                       start=True, stop=False)
                    mm(out=pre1[:, :w], lhsT=W[f"pe_W1s_{s}"][:],
                       rhs=hs3[:, 0, cs], start=False, stop=False)
                    mm(out=pre1[:, :w], lhsT=W[f"pe_W1e_{s}"][:],
                       rhs=r2_t[:, cs], start=False, stop=True)
                    a1 = sba.tile([P, 512], f16, tag="pe_a1")
                    if si % 2 == 0:
                        nc.scalar.activation(out=a1[:, :w], in_=pre1[:, :w],
                                             func=Relu,
                                             bias=W[f"pe_b1_{s}"][:, :1])
                    else:
                        nc.vector.tensor_scalar(out=a1[:, :w], in0=pre1[:, :w],
                                                scalar1=W[f"pe_b1_{s}"][:, :1],
                                                scalar2=0.0, op0=AO.add,
                                                op1=AO.max)
                    a2p = ps_b.tile([P, 512], f32, tag="mm_big")
                    mm(out=a2p[:, :w], lhsT=W[f"pe_W2_{s}"][:], rhs=a1[:, :w],
                       start=True, stop=True)
                    a2 = sba.tile([P, 512], f16, tag="pe_a2")
                    if si % 2 == 1:
                        nc.scalar.activation(out=a2[:, :w], in_=a2p[:, :w],
                                             func=Relu,
                                             bias=W[f"pe_b2_{s}"][:, :1])
                    else:
                        nc.vector.tensor_scalar(out=a2[:, :w], in0=a2p[:, :w],
                                                scalar1=W[f"pe_b2_{s}"][:, :1],
                                                scalar2=0.0, op0=AO.add,
                                                op1=AO.max)
                    m_all = ps_b.tile([P, 512], f32, tag="mm_big")
                    for j in range(k):
                        mm(out=m_all[:, j * P:(j + 1) * P],
                           lhsT=a2[:, j * P:(j + 1) * P],
                           rhs=W[f"pe_W3_{s}"][:], start=True, stop=True)
                    m16 = sba.tile([P, 512], f16, tag="m16")
                    if si % 2 == 0:
                        nc.vector.tensor_copy(m16[:, :w], m_all[:, :w])
                    else:
                        nc.scalar.activation(out=m16[:, :w], in_=m_all[:, :w],
                                             func=Copy)
                    for j in range(k):
                        t = t0 + j
                        mm(out=aggp[:], lhsT=osc[:, t * P:(t + 1) * P],
                           rhs=m16[:, j * P:(j + 1) * P],
                           start=(t == 0), stop=(t == T - 1))

                agg_sb = sbh.tile([P, L], f32, tag="agg_sb")
                nc.vector.tensor_scalar(out=agg_sb[:], in0=aggp[:],
                                        scalar1=blob_t[:, 0:1], scalar2=None,
                                        op0=AO.mult)
                b3m = sbh.tile([P, L], f32, tag="b3m")
                nc.vector.tensor_scalar(out=b3m[:], in0=b3b[:],
                                        scalar1=blob_t[:, 1:2], scalar2=None,
                                        op0=AO.mult)
                nc.vector.tensor_tensor(out=agg_sb[:], in0=agg_sb[:],
                                        in1=b3m[:], op=AO.add)
                trp = ps_s.tile([P, P], f32, tag="ps_small")
                nc.tensor.transpose(out=trp[:], in_=agg_sb[:],
                                    identity=identity[:])
                agg16 = sbh.tile([P, P], f16, tag="agg16")
                nc.vector.tensor_copy(agg16[:], trp[:])

                n1p = ps_s.tile([P, P], f32, tag="ps_small")
                mm(out=n1p[:], lhsT=W[f"pn_W1h_{s}"][:], rhs=h16[:],
                   start=True, stop=False)
                mm(out=n1p[:], lhsT=W[f"pn_W1a_{s}"][:], rhs=agg16[:],
                   start=False, stop=True)
                n1 = sbn.tile([P, P], f16, tag="n1")
                nc.vector.tensor_scalar(out=n1[:], in0=n1p[:],
                                        scalar1=W[f"pn_b1_{s}"][:, :1],
                                        scalar2=0.0, op0=AO.add, op1=AO.max)
                n2p = ps_s.tile([P, P], f32, tag="ps_small")
                mm(out=n2p[:], lhsT=W[f"pn_W2_{s}"][:], rhs=n1[:],
                   start=True, stop=True)
                n2 = sbn.tile([P, P], f16, tag="n2")
                nc.vector.tensor_scalar(out=n2[:], in0=n2p[:],
                                        scalar1=W[f"pn_b2_{s}"][:, :1],
                                        scalar2=0.0, op0=AO.add, op1=AO.max)
                n3p = ps_s.tile([P, P], f32, tag="ps_small")
                mm(out=n3p[:], lhsT=W[f"pn_W3_{s}"][:], rhs=n2[:],
                   start=True, stop=True)
                delta = sbn.tile([P, P], f32, tag="delta")
                nc.vector.tensor_scalar(out=delta[:], in0=n3p[:],
                                        scalar1=W[f"pn_b3_{s}"][:, :1],
                                        scalar2=None, op0=AO.add)
                hnew = sbh.tile([P, P], f32, tag="hnew")
                nc.vector.tensor_tensor(out=hnew[:], in0=hfm_t[:],
                                        in1=delta[:], op=AO.add)
                nc.sync.dma_start(out=h_fm[wr][ts(b, P), :], in_=hnew[:])
                if s < S - 1:
                    trh = ps_s.tile([P, P], f32, tag="ps_small")
                    nc.tensor.transpose(out=trh[:], in_=hnew[:],
                                        identity=identity[:])
                    hn16 = sbh.tile([P, P], f16, tag="hn16")
                    nc.vector.tensor_copy(hn16[:], trh[:])
                    nc.sync.dma_start(out=h_own[ts(b, P), :], in_=hn16[:])
            if s < S - 1:
                nc.gpsimd.collective_compute(
                    "AllGather", mybir.AluOpType.bypass,
                    replica_groups=[list(range(n_cores))],
                    ins=[h_own[:, :]], outs=[h_tbl[wr][:, :]])

        # ---- decoder ----
        for b in range(BPC):
            hfm_t = sbh.tile([P, P], f32, tag="dec_h")
            nc.scalar.dma_start(out=hfm_t[:], in_=h_fm[S % 2][ts(b, P), :])
            h16 = sbn.tile([P, P], f16, tag="dec_h16")
            nc.scalar.activation(out=h16[:], in_=hfm_t[:], func=Copy)
            d1p = ps_s.tile([P, P], f32, tag="ps_small")
            mm(out=d1p[:], lhsT=W["de_W1"][:], rhs=h16[:], start=True,
               stop=True)
            d1 = sbn.tile([P, P], f16, tag="dec_d1")
            nc.scalar.activation(out=d1[:], in_=d1p[:], func=Relu,
                                 bias=W["de_b1"][:, :1])
            d2p = ps_s.tile([P, P], f32, tag="ps_small")
            mm(out=d2p[:], lhsT=W["de_W2"][:], rhs=d1[:], start=True,
               stop=True)
            d2 = sbn.tile([P, P], f16, tag="dec_d2")
            nc.vector.tensor_scalar(out=d2[:], in0=d2p[:],
                                    scalar1=W["de_b2"][:, :1], scalar2=0.0,
                                    op0=AO.add, op1=AO.max)
            dp = ps_s.tile([P, OD], f32, tag="ps_small")
            mm(out=dp[:], lhsT=d2[:], rhs=W["de_W3"][:], start=True,
               stop=False)
            mm(out=dp[:], lhsT=ones_row[:], rhs=W["de_b3"][:], start=False,
               stop=True)
            osb = sbn.tile([P, OD], f32, tag="osb")
            nc.vector.tensor_copy(osb[:], dp[:])
            nc.sync.dma_start(out=out_d[ts(b, P), :], in_=osb[:])

    nc.finalize()
    return nc


def _ensure_ntff_hook():
    """Register the axon NTFF profiling hook if the image lacks
    antenv.axon_hooks (replicates trn_boot's ctypes wiring)."""
    import sys
    import types
    try:
        import antenv.axon_hooks  # noqa: F401
        return
    except ImportError:
        pass
    import contextlib
    import ctypes
    import antenv

    m = types.ModuleType("antenv.axon_hooks")
    state = {"hook": None, "tried": False}

    def set_axon_ntff_profile_hook(hook):
        state["hook"] = hook

    def _make_hook(so_path="/opt/axon/libaxon_pjrt.so"):
        lib = ctypes.CDLL(so_path)
        if not hasattr(lib, "axon_start_nrt_profile"):
            return None
        lib.axon_start_nrt_profile.argtypes = [
            ctypes.POINTER(ctypes.c_int64), ctypes.c_size_t]
        lib.axon_start_nrt_profile.restype = ctypes.c_int64
        lib.axon_stop_nrt_profile.argtypes = [ctypes.c_char_p]
        lib.axon_stop_nrt_profile.restype = ctypes.c_int64

        @contextlib.contextmanager
        def _hook(output_dir, device_ids):
            import jax
            jax.devices()
            if device_ids:
                ids = (ctypes.c_int64 * len(device_ids))(*device_ids)
                rc = lib.axon_start_nrt_profile(ids, len(device_ids))
            else:
                rc = lib.axon_start_nrt_profile(None, 0)
            if rc != 0:
                raise RuntimeError(f"axon_start_nrt_profile rc={rc}")
            try:
                yield
            finally:
                n = lib.axon_stop_nrt_profile(str(output_dir).encode())
                print(f"ntff profile: {n} file(s) written to {output_dir}")

        return _hook

    def get_axon_ntff_profile_hook():
        if state["hook"] is None and not state["tried"]:
            state["tried"] = True
            try:
                state["hook"] = _make_hook()
            except OSError:
                state["hook"] = None
        return state["hook"]

    m.set_axon_ntff_profile_hook = set_axon_ntff_profile_hook
    m.get_axon_ntff_profile_hook = get_axon_ntff_profile_hook
    sys.modules["antenv.axon_hooks"] = m
    antenv.axon_hooks = m


def kernel(**inputs):
    n_cores = 8
    params, in_maps = prep_host(inputs, n_cores)
    nc = build_program(params, debug=False)

    from concourse.bass_utils import run_bass_kernel_spmd
    import time
    trace = bool(int(os.environ.get("KERNEL_TRACE", "0")))
    if trace:
        try:
            _ensure_ntff_hook()
        except Exception:
            pass
    t0 = time.time()
    try:
        res = run_bass_kernel_spmd(nc, in_maps, list(range(n_cores)),
                                   trace=trace)
    except ModuleNotFoundError:
        res = run_bass_kernel_spmd(nc, in_maps, list(range(n_cores)),
                                   trace=False)
    LAST["wall_s"] = time.time() - t0
    LAST["exec_time_ns"] = getattr(res, "exec_time_ns", None)
    LAST["profile_json"] = getattr(res, "profile_json", None)
    LAST["params"] = params
    out = np.concatenate([r["out"] for r in res.results], axis=0)
    return np.ascontiguousarray(out[:params["N"]].astype(np.float32))
